# revision 24
# baseline (speedup 1.0000x reference)
"""Trainium2 Bass kernel for the Adaptive attention-sentinel module.

Full inputs -> data-parallel over batch B=128 across 8 NeuronCores
(16 batches/core). Each core runs an identical SPMD program on its
batch shard; outputs are concatenated on the host.

Per-core shapes (hardcoded):
  x       [16, 64, 1024]   tokens = 16*64 = 1024
  hiddens [16, 64, 512]
  cells   [16, 64, 512]
  V       [16, 49, 512]
  weights full (replicated): Wv/Wg/Ws [49,512], wh [49],
  Wx [512,1024], Wsh [512,512], Wm [10000,512], bm [10000]

Pipeline (token-major, PE transposes at matmul boundaries, bf16 matmuls):
  gate  = x @ WxT + h_prev @ WshT          -> sigmoid
  sent  = sig(gate) * tanh(cells)
  cv    = V @ WvT  (per batch, 49x49)      -> DRAM scratch, re-read broadcast
  cg    = hiddens @ WgT
  z     = sum_n wh[n] * tanh(cv + cg)      (content tile [tok, 49k, 49n])
  alpha = softmax_k(z)
  cs    = sent @ WsT + cg ; z_ext = wh . tanh(cs)
  beta  = extended-softmax last slot
  c     = alpha @ V  (pair-stacked block-diag matmul)
  out   = (beta*sent + (1-beta)*c + hiddens) @ WmT   (+ bm on host; bm==0 here)
"""

import os
import sys
from contextlib import ExitStack

import numpy as np

sys.path.insert(0, "/opt/trn_rl_repo")

import concourse.bass as bass
import concourse.tile as tile
from concourse import mybir
from concourse.masks import make_identity

F32 = mybir.dt.float32
BF16 = mybir.dt.bfloat16

# per-core dims
BC = 16          # batches per core
T = 64
NTOK = BC * T    # 1024
H = 512
E2 = 1024
KF = 49          # spatial features (and attn dim)
VOC = 10000
P = 128
HK = H // P      # 4
EK = E2 // P     # 8
MT = NTOK // P   # 8 token tiles (2 batches each)
NVT = (VOC + 511) // 512   # 20 vocab n-tiles (last = 272)
VT = (VOC + P - 1) // P    # 79 vocab p-tiles for Wm transpose (last = 16)
OCH = 2560                 # out DMA chunk (4 chunks: 2560*3 + 2320)


def _split_multi_waits(nc):
    """The staged walrus accepts at most ONE embedded sync wait per
    instruction; Tile freely emits several. Hoist the extras onto
    standalone EventSemaphore instructions on the same engine, placed
    immediately before — identical runtime semantics (the engine blocks
    on each in order)."""
    ctr = [0]
    for fn in nc.m.functions:
        for blk in fn.blocks:
            new_insts = []
            for inst in blk.instructions:
                si = inst.sync_info
                waits = list(si.on_wait) if (si is not None and si.on_wait) else []
                if len(waits) > 1:
                    for w in waits[:-1]:
                        ev = mybir.InstEventSemaphore(
                            name=f"EVSPLIT-{ctr[0]}", ins=[], outs=[],
                            sync_info=mybir.SyncInfo(on_wait=[w], on_update=[]),
                        )
                        ev.engine = inst.engine
                        new_insts.append(ev)
                        ctr[0] += 1
                    inst.sync_info = mybir.SyncInfo(
                        on_wait=[waits[-1]],
                        on_update=list(si.on_update) if si.on_update else [],
                    )
                new_insts.append(inst)
            blk.instructions[:] = new_insts
    return nc


def build_core_graph(shift=8.0):
    nc = bass.Bass()

    x_h = nc.declare_dram_parameter("x", [BC, T, E2], F32, isOutput=False)
    hid_h = nc.declare_dram_parameter("hiddens", [BC, T, H], F32, isOutput=False)
    cel_h = nc.declare_dram_parameter("cells", [BC, T, H], F32, isOutput=False)
    v_h = nc.declare_dram_parameter("V", [BC, KF, H], F32, isOutput=False)
    wv_h = nc.declare_dram_parameter("Wv", [KF, H], F32, isOutput=False)
    wg_h = nc.declare_dram_parameter("Wg", [KF, H], F32, isOutput=False)
    ws_h = nc.declare_dram_parameter("Ws", [KF, H], F32, isOutput=False)
    wh_h = nc.declare_dram_parameter("wh", [KF], F32, isOutput=False)
    wx_h = nc.declare_dram_parameter("Wx", [H, E2], F32, isOutput=False)
    wsh_h = nc.declare_dram_parameter("Wsh", [H, H], F32, isOutput=False)
    wm_h = nc.declare_dram_parameter("Wm", [VOC, H], F32, isOutput=False)

    out_h = nc.declare_dram_parameter("out", [BC, T, VOC], F32, isOutput=True)
    alpha_h = nc.declare_dram_parameter("alpha", [BC, T, KF], F32, isOutput=True)
    beta_h = nc.declare_dram_parameter("beta", [BC, T, 1], F32, isOutput=True)

    # internal DRAM scratch for cv: [pair][half][k*n] contiguous bf16
    cv_dram = nc.dram_tensor("cv_scratch", [BC // 2, 2, KF * KF], BF16)
    VOCP = 10112                       # vocab padded to x128 for xbar
    wm_bf_dram = nc.dram_tensor("wm_bf", [VOCP, H], BF16)

    x_flat = x_h[:].flatten_outer_dims()        # [1024, 1024]
    hid_flat = hid_h[:].flatten_outer_dims()    # [1024, 512]
    cel_flat = cel_h[:].flatten_outer_dims()
    v_flat = v_h[:].flatten_outer_dims()        # [784, 512]
    out_flat = out_h[:].flatten_outer_dims()    # [1024, 10000]
    alpha_flat = alpha_h[:].flatten_outer_dims()
    beta_flat = beta_h[:].flatten_outer_dims()

    with tile.TileContext(nc) as tc, ExitStack() as ctx:
        const = ctx.enter_context(tc.tile_pool(name="const", bufs=1))
        # PSUM (8 banks): ptr 2 + pgc 2 + pmain 4
        ptr = ctx.enter_context(tc.tile_pool(name="ptr", bufs=2, space="PSUM"))
        pgc = ctx.enter_context(tc.tile_pool(name="pgc", bufs=2, space="PSUM"))
        pmain = ctx.enter_context(tc.tile_pool(name="pmain", bufs=4, space="PSUM"))
        # setup-only SBUF working pool, released before the main-loop pools
        # open so its zone is reused (stack allocator)
        wctx = ExitStack()
        wpool = wctx.enter_context(tc.tile_pool(name="wpool", bufs=2))
        wpool6 = wctx.enter_context(tc.tile_pool(name="wpool6", bufs=6))

        # ---------------- constants / weights setup ----------------
        id_f32 = const.tile([P, P], F32)
        make_identity(nc, id_f32)
        id_bf = const.tile([P, P], BF16)
        nc.vector.tensor_copy(id_bf, id_f32)

        whrep = const.tile([P, KF], F32)
        wh_ap = wh_h[:]
        nc.gpsimd.dma_start(
            out=whrep,
            in_=bass.AP(tensor=wh_ap.tensor, offset=wh_ap.offset,
                        ap=[[0, P], [1, KF]]),
        )
        whrep_bf = const.tile([P, KF], BF16)
        nc.vector.tensor_copy(whrep_bf, whrep)
        negshift = const.tile([P, 1], F32)
        nc.vector.memset(negshift, -shift)

        def transpose_weight(dst, src_h, rows):
            # src [rows<=128, 512] f32 DRAM -> dst [128, 4, rows] BF16
            w_sb = wpool.tile([rows, H], F32, tag="w_sb")
            nc.sync.dma_start(out=w_sb, in_=src_h)
            ps = ptr.tile([P, 512], F32, tag="ptr")
            for kt in range(HK):
                nc.tensor.transpose(
                    ps[:, kt * P:kt * P + rows],
                    w_sb[:, kt * P:(kt + 1) * P],
                    id_f32[:rows, :rows],
                )
            nc.vector.tensor_copy(
                dst[:, :, :],
                ps.rearrange("p (k r) -> p k r", k=HK)[:, :, :rows],
            )

        # WxT [128, 8, 512] : e on partitions (8 e-tiles), h on free
        WxT = const.tile([P, EK, H], BF16)
        for ht in range(HK):
            wx_sb = wpool.tile([P, E2], F32, tag="wx_sb")
            nc.sync.dma_start(out=wx_sb, in_=wx_h[ht * P:(ht + 1) * P, :])
            for eg in range(2):          # two psum banks of 4 transposes
                ps = ptr.tile([P, 512], F32, tag="ptr")
                for j in range(4):
                    et = eg * 4 + j
                    nc.tensor.transpose(
                        ps[:, j * P:(j + 1) * P],
                        wx_sb[:, et * P:(et + 1) * P], id_f32)
                nc.vector.tensor_copy(
                    WxT[:, eg * 4:(eg + 1) * 4, ht * P:(ht + 1) * P], ps)

        # WshT [128, 4, 512]
        WshT = const.tile([P, HK, H], BF16)
        for ht in range(HK):
            wsh_sb = wpool.tile([P, H], F32, tag="wsh_sb")
            nc.sync.dma_start(out=wsh_sb, in_=wsh_h[ht * P:(ht + 1) * P, :])
            ps = ptr.tile([P, 512], F32, tag="ptr")
            for it in range(HK):
                nc.tensor.transpose(
                    ps[:, it * P:(it + 1) * P],
                    wsh_sb[:, it * P:(it + 1) * P], id_f32)
            nc.vector.tensor_copy(WshT[:, :, ht * P:(ht + 1) * P], ps)

        # WvT / WgT / WsT [128, 4, 49]
        WvT = const.tile([P, HK, KF], BF16)
        WgT = const.tile([P, HK, KF], BF16)
        WsT = const.tile([P, HK, KF], BF16)
        transpose_weight(WvT, wv_h[:, :], KF)
        transpose_weight(WgT, wg_h[:, :], KF)
        transpose_weight(WsT, ws_h[:, :], KF)

        # V2 [98, 8, 512] bf16 pair-stacked (cast during DMA)
        V2 = const.tile([2 * KF, BC // 2, H], BF16)
        for pr in range(BC // 2):
            nc.gpsimd.dma_start(
                out=V2[:, pr, :],
                in_=v_flat[pr * 2 * KF:(pr + 1) * 2 * KF, :],
            )

        # cv per pair: transpose V pairs, matmul with WvT, stash to DRAM
        cv_all = const.tile([2 * KF, BC // 2, KF], BF16)
        for pr in range(BC // 2):
            v_sb = wpool.tile([2 * KF, H], F32, tag="v_sb")
            nc.sync.dma_start(
                out=v_sb, in_=v_flat[pr * 2 * KF:(pr + 1) * 2 * KF, :])
            vT = wpool.tile([P, HK, 2 * KF], BF16, tag="vT")
            ps = ptr.tile([P, 512], F32, tag="ptr")
            for kt in range(HK):
                nc.tensor.transpose(
                    ps[:, kt * P:kt * P + 2 * KF],
                    v_sb[:, kt * P:(kt + 1) * P],
                    id_f32[:2 * KF, :2 * KF],
                )
            nc.vector.tensor_copy(
                vT[:, :, :],
                ps.rearrange("p (k r) -> p k r", k=HK)[:, :, :2 * KF],
            )
            pcv = pgc.tile([2 * KF, 512], F32, tag="pgc")
            for kt in range(HK):
                nc.tensor.matmul(
                    pcv[:, :KF], vT[:, kt, :], WvT[:, kt, :],
                    start=(kt == 0), stop=(kt == HK - 1),
                )
            nc.vector.tensor_copy(cv_all[:, pr, :], pcv[:, :KF])
        for pr in range(BC // 2):
            nc.sync.dma_start(
                out=bass.AP(tensor=cv_dram, offset=pr * 2 * KF * KF,
                            ap=[[KF * KF, 2], [KF, KF], [1, KF]]),
                in_=cv_all[:, pr, :],
            )

        # WmT [128, 4, 10000] bf16: cast-load Wm tiles, PE-transpose (bf16)
        WmT = const.tile([P, HK, VOC], BF16)
        for vt in range(VT):
            pv = min(P, VOC - vt * P)
            wm_sb = wpool6.tile([P, H], BF16, tag="wm_sb")
            nc.gpsimd.dma_start(
                out=wm_sb[:pv, :], in_=wm_h[vt * P:vt * P + pv, :])
            ps = ptr.tile([P, 512], BF16, tag="ptr")
            for kt in range(HK):
                nc.tensor.transpose(
                    ps[:, kt * P:kt * P + pv],
                    wm_sb[:pv, kt * P:(kt + 1) * P],
                    id_bf[:pv, :pv],
                )
            nc.vector.tensor_copy(
                WmT[:, :, vt * P:vt * P + pv],
                ps.rearrange("p (k r) -> p k r", k=HK)[:, :, :pv],
            )

        # release setup pool, open main-loop pools in its place
        wctx.close()
        actp = ctx.enter_context(tc.tile_pool(name="actp", bufs=3))
        work = ctx.enter_context(tc.tile_pool(name="work", bufs=2))
        trp = ctx.enter_context(tc.tile_pool(name="trp", bufs=3))
        contp = ctx.enter_context(tc.tile_pool(name="contp", bufs=2))
        smallp = ctx.enter_context(tc.tile_pool(name="smallp", bufs=3))
        outp = ctx.enter_context(tc.tile_pool(name="outp", bufs=3))

        # ---------------- main loop over token tiles ----------------
        # software-pipelined: tile m+1's "head" (loads, hT, cg, content
        # chain) is issued before tile m's main-projection burst, so its
        # transposes/psums are in flight while PE crunches the mains.

        def produce_head(m):
            R0 = m * P
            st = {}
            content = contp.tile([P, KF, KF], BF16, tag="content")
            for half in range(2):
                nc.gpsimd.dma_start(
                    out=content[half * T:(half + 1) * T, :, :],
                    in_=bass.AP(
                        tensor=cv_dram,
                        offset=(m * 2 + half) * KF * KF,
                        ap=[[0, T], [KF, KF], [1, KF]],
                    ),
                )
            x_bf = actp.tile([P, E2], BF16, tag="x_bf")
            nc.gpsimd.dma_start(out=x_bf, in_=x_flat[R0:R0 + P, :])
            hid = actp.tile([P, H], F32, tag="hid")
            nc.sync.dma_start(out=hid, in_=hid_flat[R0:R0 + P, :])
            cel = actp.tile([P, H], F32, tag="cel")
            nc.sync.dma_start(out=cel, in_=cel_flat[R0:R0 + P, :])

            h_bf = work.tile([P, H], BF16, tag="h_bf")
            nc.vector.tensor_copy(h_bf, hid)
            hT = trp.tile([P, HK, P], BF16, tag="hT")
            ps_h = ptr.tile([P, 512], BF16, tag="ptr")
            for kt in range(HK):
                nc.tensor.transpose(
                    ps_h[:, kt * P:(kt + 1) * P],
                    h_bf[:, kt * P:(kt + 1) * P], id_bf)
            nc.vector.tensor_copy(hT[:, :, :], ps_h)

            xT = trp.tile([P, EK, P], BF16, tag="xT")
            for eg in range(2):
                ps = ptr.tile([P, 512], BF16, tag="ptr")
                for j in range(4):
                    et = eg * 4 + j
                    nc.tensor.transpose(
                        ps[:, j * P:(j + 1) * P],
                        x_bf[:, et * P:(et + 1) * P], id_bf)
                nc.vector.tensor_copy(xT[:, eg * 4:(eg + 1) * 4, :], ps)

            hpT = trp.tile([P, HK, P], BF16, tag="hpT")
            for kt in range(HK):
                nc.vector.memset(hpT[:, kt, 0:1], 0.0)
                nc.vector.memset(hpT[:, kt, T:T + 1], 0.0)
                nc.vector.tensor_copy(hpT[:, kt, 1:T], hT[:, kt, 0:T - 1])
                nc.vector.tensor_copy(hpT[:, kt, T + 1:P], hT[:, kt, T:P - 1])

            # cg = hiddens@WgT  -> content chain -> zfull[:, :49]
            pcg = pgc.tile([P, 512], F32, tag="pgc")
            for kt in range(HK):
                nc.tensor.matmul(pcg[:, :KF], hT[:, kt, :], WgT[:, kt, :],
                                 start=(kt == 0), stop=(kt == HK - 1))
            cg = smallp.tile([P, KF], F32, tag="cg")
            nc.vector.tensor_copy(cg, pcg[:, :KF])
            cg_bf = smallp.tile([P, KF], BF16, tag="cg_bf")
            nc.vector.tensor_copy(cg_bf, pcg[:, :KF])

            zfull = smallp.tile([P, KF + 1], F32, tag="zfull")
            cg_b = cg_bf.unsqueeze(1).to_broadcast((P, KF, KF))
            nc.vector.tensor_tensor(content, content, cg_b,
                                    op=mybir.AluOpType.add)
            nc.scalar.activation(content, content,
                                 mybir.ActivationFunctionType.Tanh)
            wh_b = whrep_bf.unsqueeze(1).to_broadcast((P, KF, KF))
            nc.vector.tensor_tensor(content, content, wh_b,
                                    op=mybir.AluOpType.mult)
            nc.vector.reduce_sum(zfull[:, 0:KF], content,
                                 axis=mybir.AxisListType.X)

            st.update(hid=hid, cel=cel, hT=hT, xT=xT, hpT=hpT,
                      cg=cg, zfull=zfull)
            return st

        head = produce_head(0)
        for m in range(MT):
            R0 = m * P
            hid = head["hid"]; cel = head["cel"]; hT = head["hT"]
            xT = head["xT"]; hpT = head["hpT"]; cg = head["cg"]
            zfull = head["zfull"]

            pgate = pgc.tile([P, 512], F32, tag="pgc")
            nmm = EK + HK
            i = 0
            for et in range(EK):
                nc.tensor.matmul(pgate, xT[:, et, :], WxT[:, et, :],
                                 start=(i == 0), stop=(i == nmm - 1))
                i += 1
            for kt in range(HK):
                nc.tensor.matmul(pgate, hpT[:, kt, :], WshT[:, kt, :],
                                 start=(i == 0), stop=(i == nmm - 1))
                i += 1

            sig = work.tile([P, H], F32, tag="sig")
            nc.scalar.activation(sig, pgate, mybir.ActivationFunctionType.Sigmoid)
            tnc = work.tile([P, H], F32, tag="tnc")
            nc.scalar.activation(tnc, cel, mybir.ActivationFunctionType.Tanh)
            sent = work.tile([P, H], F32, tag="sent")
            nc.vector.tensor_mul(sent, sig, tnc)

            # cs = sent@WsT + cg ; z_ext = wh . tanh(cs)
            sent_bf = work.tile([P, H], BF16, tag="sent_bf")
            nc.vector.tensor_copy(sent_bf, sent)
            sT = trp.tile([P, HK, P], BF16, tag="sT")
            ps_s = ptr.tile([P, 512], BF16, tag="ptr")
            for kt in range(HK):
                nc.tensor.transpose(
                    ps_s[:, kt * P:(kt + 1) * P],
                    sent_bf[:, kt * P:(kt + 1) * P], id_bf)
            nc.vector.tensor_copy(sT[:, :, :], ps_s)
            pcs = pgc.tile([P, 512], F32, tag="pgc")
            for kt in range(HK):
                nc.tensor.matmul(pcs[:, :KF], sT[:, kt, :], WsT[:, kt, :],
                                 start=(kt == 0), stop=(kt == HK - 1))
            cs = smallp.tile([P, KF], F32, tag="cs")
            nc.vector.tensor_add(cs, pcs[:, :KF], cg)
            tcs = smallp.tile([P, KF], F32, tag="tcs")
            nc.scalar.activation(tcs, cs, mybir.ActivationFunctionType.Tanh)
            ttmp = smallp.tile([P, KF], F32, tag="ttmp")
            nc.vector.tensor_mul(ttmp, tcs, whrep)
            nc.vector.reduce_sum(zfull[:, KF:KF + 1], ttmp,
                                 axis=mybir.AxisListType.X)

            # issue next tile's head here: PE fills the softmax/c/chat
            # serial section with next-tile transposes + cg matmuls
            if m + 1 < MT:
                head = produce_head(m + 1)

            # fixed-shift exp (shift = sum|wh|+1 bound, baked at build):
            # e49 feeds the c-matmul without waiting on the sentinel chain
            e49 = smallp.tile([P, KF], F32, tag="e49")
            s1 = smallp.tile([P, 1], F32, tag="s1")
            nc.scalar.activation(e49, zfull[:, 0:KF],
                                 mybir.ActivationFunctionType.Exp,
                                 bias=negshift, accum_out=s1)
            rs1 = smallp.tile([P, 1], F32, tag="rs1")
            nc.vector.reciprocal(rs1, s1)
            alpha_sb = smallp.tile([P, KF], F32, tag="alpha_sb")
            nc.vector.tensor_scalar_mul(alpha_sb, e49, rs1)
            nc.sync.dma_start(out=alpha_flat[R0:R0 + P, :], in_=alpha_sb)

            e_ext = smallp.tile([P, 1], F32, tag="e_ext")
            nc.scalar.activation(e_ext, zfull[:, KF:KF + 1],
                                 mybir.ActivationFunctionType.Exp,
                                 bias=negshift)
            den = smallp.tile([P, 1], F32, tag="den")
            nc.vector.tensor_add(den, s1, e_ext)
            rden = smallp.tile([P, 1], F32, tag="rden")
            nc.vector.reciprocal(rden, den)
            beta_sb = smallp.tile([P, 1], F32, tag="beta_sb")
            nc.vector.tensor_mul(beta_sb, e_ext, rden)
            nc.sync.dma_start(out=beta_flat[R0:R0 + P, :], in_=beta_sb)

            # c*s1 = e49 @ V  via block-diag pair trick
            a2 = smallp.tile([P, 2 * KF], F32, tag="a2")
            nc.vector.memset(a2, 0.0)
            nc.vector.tensor_copy(a2[0:T, 0:KF], e49[0:T, :])
            nc.vector.tensor_copy(a2[T:P, KF:2 * KF], e49[T:P, :])
            psA = ptr.tile([P, 512], F32, tag="ptr")
            nc.tensor.transpose(psA[:2 * KF, :P], a2, id_f32)
            a2T = trp.tile([2 * KF, P], BF16, tag="a2T")
            nc.vector.tensor_copy(a2T, psA[:2 * KF, :P])
            pc = pgc.tile([P, 512], F32, tag="pgc")
            nc.tensor.matmul(pc, a2T, V2[:, m, :], start=True, stop=True)

            # a = (1-beta)*(pc*rs1) + beta*sent + hid, fused:
            # rob = rs1*(1-beta); t1 = pc*rob + hid; a = sent*beta + t1
            omb = smallp.tile([P, 1], F32, tag="omb")
            nc.vector.tensor_scalar(out=omb, in0=beta_sb, scalar1=-1.0,
                                    scalar2=1.0, op0=mybir.AluOpType.mult,
                                    op1=mybir.AluOpType.add)
            rob = smallp.tile([P, 1], F32, tag="rob")
            nc.vector.tensor_mul(rob, rs1, omb)
            t1 = work.tile([P, H], F32, tag="t1")
            nc.vector.scalar_tensor_tensor(
                out=t1, in0=pc, scalar=rob, in1=hid,
                op0=mybir.AluOpType.mult, op1=mybir.AluOpType.add,
            )
            a_bf = work.tile([P, H], BF16, tag="a_bf")
            nc.vector.scalar_tensor_tensor(
                out=a_bf, in0=sent, scalar=beta_sb, in1=t1,
                op0=mybir.AluOpType.mult, op1=mybir.AluOpType.add,
            )
            aT = trp.tile([P, HK, P], BF16, tag="aT")
            ps_a = ptr.tile([P, 512], BF16, tag="ptr")
            for kt in range(HK):
                nc.tensor.transpose(
                    ps_a[:, kt * P:(kt + 1) * P],
                    a_bf[:, kt * P:(kt + 1) * P], id_bf)
            nc.vector.tensor_copy(aT[:, :, :], ps_a)

            # out = a @ WmT ; evacs alternate DVE/ACT, 4 big DMAs
            for oc in range(4):
                base = oc * OCH
                width = min(OCH, VOC - base)
                osb = outp.tile([P, OCH], F32, tag="osb")
                nsub = (width + 511) // 512
                for s in range(nsub):
                    nb = base + s * 512
                    w = min(512, VOC - nb)
                    po = pmain.tile([P, 512], F32, tag="pmain")
                    for kt in range(HK):
                        nc.tensor.matmul(
                            po[:, :w], aT[:, kt, :],
                            WmT[:, kt, nb:nb + w],
                            start=(kt == 0), stop=(kt == HK - 1),
                        )
                    if s % 2 == 0:
                        nc.vector.tensor_copy(osb[:, s * 512:s * 512 + w],
                                              po[:, :w])
                    else:
                        nc.scalar.copy(osb[:, s * 512:s * 512 + w], po[:, :w])
                nc.sync.dma_start(
                    out=out_flat[R0:R0 + P, base:base + width],
                    in_=osb[:, :width],
                )

    _split_multi_waits(nc)
    return nc


_GRAPH_CACHE = {}


def _get_graph(shift):
    key = round(float(shift), 3)
    if key not in _GRAPH_CACHE:
        _GRAPH_CACHE[key] = build_core_graph(key)
    return _GRAPH_CACHE[key]


def kernel(x, hiddens, cells, V, Wv, Wg, Ws, wh, Wx, Wsh, Wm, bm):
    from concourse.bass_utils import run_bass_kernel_spmd

    n_cores = 8
    B = x.shape[0]
    bc = B // n_cores

    weights = {
        "Wv": np.ascontiguousarray(Wv, np.float32),
        "Wg": np.ascontiguousarray(Wg, np.float32),
        "Ws": np.ascontiguousarray(Ws, np.float32),
        "wh": np.ascontiguousarray(wh, np.float32),
        "Wx": np.ascontiguousarray(Wx, np.float32),
        "Wsh": np.ascontiguousarray(Wsh, np.float32),
        "Wm": np.ascontiguousarray(Wm, np.float32),
    }
    in_maps = []
    for i in range(n_cores):
        sl = slice(i * bc, (i + 1) * bc)
        m = {
            "x": np.ascontiguousarray(x[sl], np.float32),
            "hiddens": np.ascontiguousarray(hiddens[sl], np.float32),
            "cells": np.ascontiguousarray(cells[sl], np.float32),
            "V": np.ascontiguousarray(V[sl], np.float32),
        }
        m.update(weights)
        in_maps.append(m)

    shift = float(np.abs(np.asarray(wh, np.float64)).sum()) + 1.0
    nc = _get_graph(shift)
    trace = bool(int(os.environ.get("KERNEL_TRACE", "0")))
    res = run_bass_kernel_spmd(nc, in_maps, core_ids=list(range(n_cores)),
                               trace=trace)
    if trace:
        kernel.last_exec_time_ns = res.exec_time_ns
        kernel.last_profile = res

    out = np.concatenate([r["out"] for r in res.results], axis=0)
    alpha = np.concatenate([r["alpha"] for r in res.results], axis=0)
    beta = np.concatenate([r["beta"] for r in res.results], axis=0)
    if np.any(bm):
        out = out + np.asarray(bm, np.float32)
    return out, alpha, beta


# revision 25
# speedup vs baseline: 1.2258x; 1.2258x over previous
"""Trainium2 Bass kernel for the Adaptive attention-sentinel module.

Full inputs -> data-parallel over batch B=128 across 8 NeuronCores
(16 batches/core). Each core runs an identical SPMD program on its
batch shard; outputs are concatenated on the host.

Per-core shapes (hardcoded):
  x       [16, 64, 1024]   tokens = 16*64 = 1024
  hiddens [16, 64, 512]
  cells   [16, 64, 512]
  V       [16, 49, 512]
  weights full (replicated): Wv/Wg/Ws [49,512], wh [49],
  Wx [512,1024], Wsh [512,512], Wm [10000,512], bm [10000]

Pipeline (token-major, PE transposes at matmul boundaries, bf16 matmuls):
  gate  = x @ WxT + h_prev @ WshT          -> sigmoid
  sent  = sig(gate) * tanh(cells)
  cv    = V @ WvT  (per batch, 49x49)      -> DRAM scratch, re-read broadcast
  cg    = hiddens @ WgT
  z     = sum_n wh[n] * tanh(cv + cg)      (content tile [tok, 49k, 49n])
  alpha = softmax_k(z)
  cs    = sent @ WsT + cg ; z_ext = wh . tanh(cs)
  beta  = extended-softmax last slot
  c     = alpha @ V  (pair-stacked block-diag matmul)
  out   = (beta*sent + (1-beta)*c + hiddens) @ WmT   (+ bm on host; bm==0 here)
"""

import os
import sys
from contextlib import ExitStack

import numpy as np

sys.path.insert(0, "/opt/trn_rl_repo")

import concourse.bass as bass
import concourse.tile as tile
from concourse import mybir
from concourse.masks import make_identity

F32 = mybir.dt.float32
BF16 = mybir.dt.bfloat16

# per-core dims
BC = 16          # batches per core
T = 64
NTOK = BC * T    # 1024
H = 512
E2 = 1024
KF = 49          # spatial features (and attn dim)
VOC = 10000
P = 128
HK = H // P      # 4
EK = E2 // P     # 8
MT = NTOK // P   # 8 token tiles (2 batches each)
NVT = (VOC + 511) // 512   # 20 vocab n-tiles (last = 272)
VT = (VOC + P - 1) // P    # 79 vocab p-tiles for Wm transpose (last = 16)
OCH = 2560                 # out DMA chunk (4 chunks: 2560*3 + 2320)


def _split_multi_waits(nc):
    """The staged walrus accepts at most ONE embedded sync wait per
    instruction; Tile freely emits several. Hoist the extras onto
    standalone EventSemaphore instructions on the same engine, placed
    immediately before — identical runtime semantics (the engine blocks
    on each in order)."""
    ctr = [0]
    for fn in nc.m.functions:
        for blk in fn.blocks:
            new_insts = []
            for inst in blk.instructions:
                si = inst.sync_info
                waits = list(si.on_wait) if (si is not None and si.on_wait) else []
                if len(waits) > 1:
                    for w in waits[:-1]:
                        ev = mybir.InstEventSemaphore(
                            name=f"EVSPLIT-{ctr[0]}", ins=[], outs=[],
                            sync_info=mybir.SyncInfo(on_wait=[w], on_update=[]),
                        )
                        ev.engine = inst.engine
                        new_insts.append(ev)
                        ctr[0] += 1
                    inst.sync_info = mybir.SyncInfo(
                        on_wait=[waits[-1]],
                        on_update=list(si.on_update) if si.on_update else [],
                    )
                new_insts.append(inst)
            blk.instructions[:] = new_insts
    return nc


def build_core_graph(shift=8.0):
    nc = bass.Bass()

    x_h = nc.declare_dram_parameter("x", [BC, T, E2], F32, isOutput=False)
    hid_h = nc.declare_dram_parameter("hiddens", [BC, T, H], F32, isOutput=False)
    cel_h = nc.declare_dram_parameter("cells", [BC, T, H], F32, isOutput=False)
    v_h = nc.declare_dram_parameter("V", [BC, KF, H], F32, isOutput=False)
    wv_h = nc.declare_dram_parameter("Wv", [KF, H], F32, isOutput=False)
    wg_h = nc.declare_dram_parameter("Wg", [KF, H], F32, isOutput=False)
    ws_h = nc.declare_dram_parameter("Ws", [KF, H], F32, isOutput=False)
    wh_h = nc.declare_dram_parameter("wh", [KF], F32, isOutput=False)
    wx_h = nc.declare_dram_parameter("Wx", [H, E2], F32, isOutput=False)
    wsh_h = nc.declare_dram_parameter("Wsh", [H, H], F32, isOutput=False)
    wm_h = nc.declare_dram_parameter("Wm", [VOC, H], F32, isOutput=False)

    out_h = nc.declare_dram_parameter("out", [BC, T, VOC], F32, isOutput=True)
    alpha_h = nc.declare_dram_parameter("alpha", [BC, T, KF], F32, isOutput=True)
    beta_h = nc.declare_dram_parameter("beta", [BC, T, 1], F32, isOutput=True)

    # internal DRAM scratch for cv: [pair][half][k*n] contiguous bf16
    cv_dram = nc.dram_tensor("cv_scratch", [BC // 2, 2, KF * KF], BF16)
    VOCP = 10112                       # vocab padded to x128 for xbar
    wm_bf_dram = nc.dram_tensor("wm_bf", [VOCP, H], BF16)

    x_flat = x_h[:].flatten_outer_dims()        # [1024, 1024]
    hid_flat = hid_h[:].flatten_outer_dims()    # [1024, 512]
    cel_flat = cel_h[:].flatten_outer_dims()
    v_flat = v_h[:].flatten_outer_dims()        # [784, 512]
    out_flat = out_h[:].flatten_outer_dims()    # [1024, 10000]
    alpha_flat = alpha_h[:].flatten_outer_dims()
    beta_flat = beta_h[:].flatten_outer_dims()

    with tile.TileContext(nc) as tc, ExitStack() as ctx:
        const = ctx.enter_context(tc.tile_pool(name="const", bufs=1))
        # PSUM (8 banks): ptr 2 + pgc 2 + pmain 4
        ptr = ctx.enter_context(tc.tile_pool(name="ptr", bufs=2, space="PSUM"))
        pgc = ctx.enter_context(tc.tile_pool(name="pgc", bufs=2, space="PSUM"))
        pmain = ctx.enter_context(tc.tile_pool(name="pmain", bufs=4, space="PSUM"))
        # setup-only SBUF working pool, released before the main-loop pools
        # open so its zone is reused (stack allocator)
        wctx = ExitStack()
        wpool = wctx.enter_context(tc.tile_pool(name="wpool", bufs=2))
        wpool6 = wctx.enter_context(tc.tile_pool(name="wpool6", bufs=6))

        # ---------------- constants / weights setup ----------------
        id_f32 = const.tile([P, P], F32)
        make_identity(nc, id_f32)
        id_bf = const.tile([P, P], BF16)
        nc.vector.tensor_copy(id_bf, id_f32)

        # tile-0 input loads first so they lead the DMA queues
        pre_x0 = const.tile([P, E2], BF16)
        nc.gpsimd.dma_start(out=pre_x0, in_=x_flat[0:P, :])
        pre_hid0 = const.tile([P, H], F32)
        nc.sync.dma_start(out=pre_hid0, in_=hid_flat[0:P, :])
        pre_cel0 = const.tile([P, H], F32)
        nc.sync.dma_start(out=pre_cel0, in_=cel_flat[0:P, :])

        whrep = const.tile([P, KF], F32)
        wh_ap = wh_h[:]
        nc.gpsimd.dma_start(
            out=whrep,
            in_=bass.AP(tensor=wh_ap.tensor, offset=wh_ap.offset,
                        ap=[[0, P], [1, KF]]),
        )
        whrep_bf = const.tile([P, KF], BF16)
        nc.vector.tensor_copy(whrep_bf, whrep)
        negshift = const.tile([P, 1], F32)
        nc.vector.memset(negshift, -shift)

        def transpose_weight(dst, src_h, rows):
            # src [rows<=128, 512] f32 DRAM -> dst [128, 4, rows] BF16
            w_sb = wpool.tile([rows, H], F32, tag="w_sb")
            nc.sync.dma_start(out=w_sb, in_=src_h)
            ps = ptr.tile([P, 512], F32, tag="ptr")
            for kt in range(HK):
                nc.tensor.transpose(
                    ps[:, kt * P:kt * P + rows],
                    w_sb[:, kt * P:(kt + 1) * P],
                    id_f32[:rows, :rows],
                )
            nc.vector.tensor_copy(
                dst[:, :, :],
                ps.rearrange("p (k r) -> p k r", k=HK)[:, :, :rows],
            )

        # WxT [128, 8, 512] : e on partitions (8 e-tiles), h on free
        WxT = const.tile([P, EK, H], BF16)
        for ht in range(HK):
            wx_sb = wpool.tile([P, E2], F32, tag="wx_sb")
            nc.sync.dma_start(out=wx_sb, in_=wx_h[ht * P:(ht + 1) * P, :])
            for eg in range(2):          # two psum banks of 4 transposes
                ps = ptr.tile([P, 512], F32, tag="ptr")
                for j in range(4):
                    et = eg * 4 + j
                    nc.tensor.transpose(
                        ps[:, j * P:(j + 1) * P],
                        wx_sb[:, et * P:(et + 1) * P], id_f32)
                nc.vector.tensor_copy(
                    WxT[:, eg * 4:(eg + 1) * 4, ht * P:(ht + 1) * P], ps)

        # WshT [128, 4, 512]
        WshT = const.tile([P, HK, H], BF16)
        for ht in range(HK):
            wsh_sb = wpool.tile([P, H], F32, tag="wsh_sb")
            nc.sync.dma_start(out=wsh_sb, in_=wsh_h[ht * P:(ht + 1) * P, :])
            ps = ptr.tile([P, 512], F32, tag="ptr")
            for it in range(HK):
                nc.tensor.transpose(
                    ps[:, it * P:(it + 1) * P],
                    wsh_sb[:, it * P:(it + 1) * P], id_f32)
            nc.vector.tensor_copy(WshT[:, :, ht * P:(ht + 1) * P], ps)

        # WvT / WgT / WsT [128, 4, 49]
        WvT = const.tile([P, HK, KF], BF16)
        WgT = const.tile([P, HK, KF], BF16)
        WsT = const.tile([P, HK, KF], BF16)
        transpose_weight(WvT, wv_h[:, :], KF)
        transpose_weight(WgT, wg_h[:, :], KF)
        transpose_weight(WsT, ws_h[:, :], KF)

        # V2 [98, 8, 512] bf16 pair-stacked (cast during DMA)
        V2 = const.tile([2 * KF, BC // 2, H], BF16)
        for pr in range(BC // 2):
            nc.gpsimd.dma_start(
                out=V2[:, pr, :],
                in_=v_flat[pr * 2 * KF:(pr + 1) * 2 * KF, :],
            )

        # cv per pair: transpose V pairs, matmul with WvT, stash to DRAM
        cv_all = const.tile([2 * KF, BC // 2, KF], BF16)
        for pr in range(BC // 2):
            v_sb = wpool.tile([2 * KF, H], F32, tag="v_sb")
            nc.sync.dma_start(
                out=v_sb, in_=v_flat[pr * 2 * KF:(pr + 1) * 2 * KF, :])
            vT = wpool.tile([P, HK, 2 * KF], BF16, tag="vT")
            ps = ptr.tile([P, 512], F32, tag="ptr")
            for kt in range(HK):
                nc.tensor.transpose(
                    ps[:, kt * P:kt * P + 2 * KF],
                    v_sb[:, kt * P:(kt + 1) * P],
                    id_f32[:2 * KF, :2 * KF],
                )
            nc.vector.tensor_copy(
                vT[:, :, :],
                ps.rearrange("p (k r) -> p k r", k=HK)[:, :, :2 * KF],
            )
            pcv = pgc.tile([2 * KF, 512], F32, tag="pgc")
            for kt in range(HK):
                nc.tensor.matmul(
                    pcv[:, :KF], vT[:, kt, :], WvT[:, kt, :],
                    start=(kt == 0), stop=(kt == HK - 1),
                )
            nc.vector.tensor_copy(cv_all[:, pr, :], pcv[:, :KF])
        for pr in range(BC // 2):
            nc.sync.dma_start(
                out=bass.AP(tensor=cv_dram, offset=pr * 2 * KF * KF,
                            ap=[[KF * KF, 2], [KF, KF], [1, KF]]),
                in_=cv_all[:, pr, :],
            )

        # WmT [128, 4, 10000] bf16: cast-load Wm tiles, PE-transpose (bf16)
        WmT = const.tile([P, HK, VOC], BF16)
        for vt in range(VT):
            pv = min(P, VOC - vt * P)
            wm_sb = wpool6.tile([P, H], BF16, tag="wm_sb")
            nc.gpsimd.dma_start(
                out=wm_sb[:pv, :], in_=wm_h[vt * P:vt * P + pv, :])
            ps = ptr.tile([P, 512], BF16, tag="ptr")
            for kt in range(HK):
                nc.tensor.transpose(
                    ps[:, kt * P:kt * P + pv],
                    wm_sb[:pv, kt * P:(kt + 1) * P],
                    id_bf[:pv, :pv],
                )
            nc.vector.tensor_copy(
                WmT[:, :, vt * P:vt * P + pv],
                ps.rearrange("p (k r) -> p k r", k=HK)[:, :, :pv],
            )

        # release setup pool, open main-loop pools in its place
        wctx.close()
        actp = ctx.enter_context(tc.tile_pool(name="actp", bufs=3))
        work = ctx.enter_context(tc.tile_pool(name="work", bufs=2))
        trp = ctx.enter_context(tc.tile_pool(name="trp", bufs=3))
        contp = ctx.enter_context(tc.tile_pool(name="contp", bufs=2))
        smallp = ctx.enter_context(tc.tile_pool(name="smallp", bufs=3))
        outp = ctx.enter_context(tc.tile_pool(name="outp", bufs=4))

        # ---------------- main loop over token tiles ----------------
        # software-pipelined: tile m+1's "head" (loads, hT, cg, content
        # chain) is issued before tile m's main-projection burst, so its
        # transposes/psums are in flight while PE crunches the mains.

        def produce_head(m):
            R0 = m * P
            st = {}
            content = contp.tile([P, KF, KF], BF16, tag="content")
            for half in range(2):
                nc.gpsimd.dma_start(
                    out=content[half * T:(half + 1) * T, :, :],
                    in_=bass.AP(
                        tensor=cv_dram,
                        offset=(m * 2 + half) * KF * KF,
                        ap=[[0, T], [KF, KF], [1, KF]],
                    ),
                )
            if m == 0:
                x_bf, hid, cel = pre_x0, pre_hid0, pre_cel0
            else:
                x_bf = actp.tile([P, E2], BF16, tag="x_bf")
                nc.gpsimd.dma_start(out=x_bf, in_=x_flat[R0:R0 + P, :])
                hid = actp.tile([P, H], F32, tag="hid")
                nc.sync.dma_start(out=hid, in_=hid_flat[R0:R0 + P, :])
                cel = actp.tile([P, H], F32, tag="cel")
                nc.sync.dma_start(out=cel, in_=cel_flat[R0:R0 + P, :])

            h_bf = work.tile([P, H], BF16, tag="h_bf")
            nc.vector.tensor_copy(h_bf, hid)
            hT = trp.tile([P, HK, P], BF16, tag="hT")
            ps_h = ptr.tile([P, 512], BF16, tag="ptr")
            for kt in range(HK):
                nc.tensor.transpose(
                    ps_h[:, kt * P:(kt + 1) * P],
                    h_bf[:, kt * P:(kt + 1) * P], id_bf)
            nc.vector.tensor_copy(hT[:, :, :], ps_h)

            xT = trp.tile([P, EK, P], BF16, tag="xT")
            for eg in range(2):
                ps = ptr.tile([P, 512], BF16, tag="ptr")
                for j in range(4):
                    et = eg * 4 + j
                    nc.tensor.transpose(
                        ps[:, j * P:(j + 1) * P],
                        x_bf[:, et * P:(et + 1) * P], id_bf)
                nc.vector.tensor_copy(xT[:, eg * 4:(eg + 1) * 4, :], ps)

            hpT = trp.tile([P, HK, P], BF16, tag="hpT")
            for kt in range(HK):
                nc.vector.memset(hpT[:, kt, 0:1], 0.0)
                nc.vector.memset(hpT[:, kt, T:T + 1], 0.0)
                nc.vector.tensor_copy(hpT[:, kt, 1:T], hT[:, kt, 0:T - 1])
                nc.vector.tensor_copy(hpT[:, kt, T + 1:P], hT[:, kt, T:P - 1])

            # cg = hiddens@WgT  -> content chain -> zfull[:, :49]
            pcg = pgc.tile([P, 512], F32, tag="pgc")
            for kt in range(HK):
                nc.tensor.matmul(pcg[:, :KF], hT[:, kt, :], WgT[:, kt, :],
                                 start=(kt == 0), stop=(kt == HK - 1))
            cg = smallp.tile([P, KF], F32, tag="cg")
            nc.vector.tensor_copy(cg, pcg[:, :KF])
            cg_bf = smallp.tile([P, KF], BF16, tag="cg_bf")
            nc.vector.tensor_copy(cg_bf, pcg[:, :KF])

            zfull = smallp.tile([P, KF + 1], F32, tag="zfull")
            cg_b = cg_bf.unsqueeze(1).to_broadcast((P, KF, KF))
            nc.vector.tensor_tensor(content, content, cg_b,
                                    op=mybir.AluOpType.add)
            nc.scalar.activation(content, content,
                                 mybir.ActivationFunctionType.Tanh)
            wh_b = whrep_bf.unsqueeze(1).to_broadcast((P, KF, KF))
            nc.vector.tensor_tensor(content, content, wh_b,
                                    op=mybir.AluOpType.mult)
            nc.vector.reduce_sum(zfull[:, 0:KF], content,
                                 axis=mybir.AxisListType.X)

            st.update(hid=hid, cel=cel, hT=hT, xT=xT, hpT=hpT,
                      cg=cg, zfull=zfull)
            return st

        head = produce_head(0)
        for m in range(MT):
            R0 = m * P
            hid = head["hid"]; cel = head["cel"]; hT = head["hT"]
            xT = head["xT"]; hpT = head["hpT"]; cg = head["cg"]
            zfull = head["zfull"]

            pgate = pgc.tile([P, 512], F32, tag="pgc")
            nmm = EK + HK
            i = 0
            for et in range(EK):
                nc.tensor.matmul(pgate, xT[:, et, :], WxT[:, et, :],
                                 start=(i == 0), stop=(i == nmm - 1))
                i += 1
            for kt in range(HK):
                nc.tensor.matmul(pgate, hpT[:, kt, :], WshT[:, kt, :],
                                 start=(i == 0), stop=(i == nmm - 1))
                i += 1

            sig = work.tile([P, H], F32, tag="sig")
            nc.scalar.activation(sig, pgate, mybir.ActivationFunctionType.Sigmoid)
            tnc = work.tile([P, H], F32, tag="tnc")
            nc.scalar.activation(tnc, cel, mybir.ActivationFunctionType.Tanh)
            sent = work.tile([P, H], F32, tag="sent")
            nc.vector.tensor_mul(sent, sig, tnc)

            # cs = sent@WsT + cg ; z_ext = wh . tanh(cs)
            sent_bf = work.tile([P, H], BF16, tag="sent_bf")
            nc.vector.tensor_copy(sent_bf, sent)
            sT = trp.tile([P, HK, P], BF16, tag="sT")
            ps_s = ptr.tile([P, 512], BF16, tag="ptr")
            for kt in range(HK):
                nc.tensor.transpose(
                    ps_s[:, kt * P:(kt + 1) * P],
                    sent_bf[:, kt * P:(kt + 1) * P], id_bf)
            nc.vector.tensor_copy(sT[:, :, :], ps_s)
            pcs = pgc.tile([P, 512], F32, tag="pgc")
            for kt in range(HK):
                nc.tensor.matmul(pcs[:, :KF], sT[:, kt, :], WsT[:, kt, :],
                                 start=(kt == 0), stop=(kt == HK - 1))
            cs = smallp.tile([P, KF], F32, tag="cs")
            nc.vector.tensor_add(cs, pcs[:, :KF], cg)
            tcs = smallp.tile([P, KF], F32, tag="tcs")
            nc.scalar.activation(tcs, cs, mybir.ActivationFunctionType.Tanh)
            ttmp = smallp.tile([P, KF], F32, tag="ttmp")
            nc.vector.tensor_mul(ttmp, tcs, whrep)
            nc.vector.reduce_sum(zfull[:, KF:KF + 1], ttmp,
                                 axis=mybir.AxisListType.X)

            # issue next tile's head here: PE fills the softmax/c/chat
            # serial section with next-tile transposes + cg matmuls
            if m + 1 < MT:
                head = produce_head(m + 1)

            # fixed-shift exp (shift = sum|wh|+1 bound, baked at build):
            # e49 feeds the c-matmul without waiting on the sentinel chain
            e49 = smallp.tile([P, KF], F32, tag="e49")
            s1 = smallp.tile([P, 1], F32, tag="s1")
            nc.scalar.activation(e49, zfull[:, 0:KF],
                                 mybir.ActivationFunctionType.Exp,
                                 bias=negshift, accum_out=s1)
            rs1 = smallp.tile([P, 1], F32, tag="rs1")
            nc.vector.reciprocal(rs1, s1)
            alpha_sb = smallp.tile([P, KF], F32, tag="alpha_sb")
            nc.vector.tensor_scalar_mul(alpha_sb, e49, rs1)
            nc.sync.dma_start(out=alpha_flat[R0:R0 + P, :], in_=alpha_sb)

            e_ext = smallp.tile([P, 1], F32, tag="e_ext")
            nc.scalar.activation(e_ext, zfull[:, KF:KF + 1],
                                 mybir.ActivationFunctionType.Exp,
                                 bias=negshift)
            den = smallp.tile([P, 1], F32, tag="den")
            nc.vector.tensor_add(den, s1, e_ext)
            rden = smallp.tile([P, 1], F32, tag="rden")
            nc.vector.reciprocal(rden, den)
            beta_sb = smallp.tile([P, 1], F32, tag="beta_sb")
            nc.vector.tensor_mul(beta_sb, e_ext, rden)
            nc.sync.dma_start(out=beta_flat[R0:R0 + P, :], in_=beta_sb)

            # c*s1 = e49 @ V  via block-diag pair trick
            a2 = smallp.tile([P, 2 * KF], F32, tag="a2")
            nc.vector.memset(a2, 0.0)
            nc.vector.tensor_copy(a2[0:T, 0:KF], e49[0:T, :])
            nc.vector.tensor_copy(a2[T:P, KF:2 * KF], e49[T:P, :])
            psA = ptr.tile([P, 512], F32, tag="ptr")
            nc.tensor.transpose(psA[:2 * KF, :P], a2, id_f32)
            a2T = trp.tile([2 * KF, P], BF16, tag="a2T")
            nc.vector.tensor_copy(a2T, psA[:2 * KF, :P])
            pc = pgc.tile([P, 512], F32, tag="pgc")
            nc.tensor.matmul(pc, a2T, V2[:, m, :], start=True, stop=True)

            # a = (1-beta)*(pc*rs1) + beta*sent + hid, fused:
            # rob = rs1*(1-beta); t1 = pc*rob + hid; a = sent*beta + t1
            omb = smallp.tile([P, 1], F32, tag="omb")
            nc.vector.tensor_scalar(out=omb, in0=beta_sb, scalar1=-1.0,
                                    scalar2=1.0, op0=mybir.AluOpType.mult,
                                    op1=mybir.AluOpType.add)
            rob = smallp.tile([P, 1], F32, tag="rob")
            nc.vector.tensor_mul(rob, rs1, omb)
            t1 = work.tile([P, H], F32, tag="t1")
            nc.vector.scalar_tensor_tensor(
                out=t1, in0=pc, scalar=rob, in1=hid,
                op0=mybir.AluOpType.mult, op1=mybir.AluOpType.add,
            )
            a_bf = work.tile([P, H], BF16, tag="a_bf")
            nc.vector.scalar_tensor_tensor(
                out=a_bf, in0=sent, scalar=beta_sb, in1=t1,
                op0=mybir.AluOpType.mult, op1=mybir.AluOpType.add,
            )
            aT = trp.tile([P, HK, P], BF16, tag="aT")
            ps_a = ptr.tile([P, 512], BF16, tag="ptr")
            for kt in range(HK):
                nc.tensor.transpose(
                    ps_a[:, kt * P:(kt + 1) * P],
                    a_bf[:, kt * P:(kt + 1) * P], id_bf)
            nc.vector.tensor_copy(aT[:, :, :], ps_a)

            # out = a @ WmT ; evacs alternate DVE/ACT, 7 mid-size DMAs
            for oc in range(7):
                base = oc * 1536
                width = min(1536, VOC - base)
                osb = outp.tile([P, 1536], F32, tag="osb")
                nsub = (width + 511) // 512
                for s in range(nsub):
                    nb = base + s * 512
                    w = min(512, VOC - nb)
                    po = pmain.tile([P, 512], F32, tag="pmain")
                    for kt in range(HK):
                        nc.tensor.matmul(
                            po[:, :w], aT[:, kt, :],
                            WmT[:, kt, nb:nb + w],
                            start=(kt == 0), stop=(kt == HK - 1),
                        )
                    if s % 2 == 0:
                        nc.vector.tensor_copy(osb[:, s * 512:s * 512 + w],
                                              po[:, :w])
                    else:
                        nc.scalar.copy(osb[:, s * 512:s * 512 + w], po[:, :w])
                nc.sync.dma_start(
                    out=out_flat[R0:R0 + P, base:base + width],
                    in_=osb[:, :width],
                )

    _split_multi_waits(nc)
    return nc


_GRAPH_CACHE = {}


def _get_graph(shift):
    key = round(float(shift), 3)
    if key not in _GRAPH_CACHE:
        _GRAPH_CACHE[key] = build_core_graph(key)
    return _GRAPH_CACHE[key]


def kernel(x, hiddens, cells, V, Wv, Wg, Ws, wh, Wx, Wsh, Wm, bm):
    from concourse.bass_utils import run_bass_kernel_spmd

    n_cores = 8
    B = x.shape[0]
    bc = B // n_cores

    weights = {
        "Wv": np.ascontiguousarray(Wv, np.float32),
        "Wg": np.ascontiguousarray(Wg, np.float32),
        "Ws": np.ascontiguousarray(Ws, np.float32),
        "wh": np.ascontiguousarray(wh, np.float32),
        "Wx": np.ascontiguousarray(Wx, np.float32),
        "Wsh": np.ascontiguousarray(Wsh, np.float32),
        "Wm": np.ascontiguousarray(Wm, np.float32),
    }
    in_maps = []
    for i in range(n_cores):
        sl = slice(i * bc, (i + 1) * bc)
        m = {
            "x": np.ascontiguousarray(x[sl], np.float32),
            "hiddens": np.ascontiguousarray(hiddens[sl], np.float32),
            "cells": np.ascontiguousarray(cells[sl], np.float32),
            "V": np.ascontiguousarray(V[sl], np.float32),
        }
        m.update(weights)
        in_maps.append(m)

    shift = float(np.abs(np.asarray(wh, np.float64)).sum()) + 1.0
    nc = _get_graph(shift)
    trace = bool(int(os.environ.get("KERNEL_TRACE", "0")))
    res = run_bass_kernel_spmd(nc, in_maps, core_ids=list(range(n_cores)),
                               trace=trace)
    if trace:
        kernel.last_exec_time_ns = res.exec_time_ns
        kernel.last_profile = res

    out = np.concatenate([r["out"] for r in res.results], axis=0)
    alpha = np.concatenate([r["alpha"] for r in res.results], axis=0)
    beta = np.concatenate([r["beta"] for r in res.results], axis=0)
    if np.any(bm):
        out = out + np.asarray(bm, np.float32)
    return out, alpha, beta


# revision 27
# speedup vs baseline: 1.2348x; 1.0074x over previous
"""Trainium2 Bass kernel for the Adaptive attention-sentinel module.

Full inputs -> data-parallel over batch B=128 across 8 NeuronCores
(16 batches/core). Each core runs an identical SPMD program on its
batch shard; outputs are concatenated on the host.

Per-core shapes (hardcoded):
  x       [16, 64, 1024]   tokens = 16*64 = 1024
  hiddens [16, 64, 512]
  cells   [16, 64, 512]
  V       [16, 49, 512]
  weights full (replicated): Wv/Wg/Ws [49,512], wh [49],
  Wx [512,1024], Wsh [512,512], Wm [10000,512], bm [10000]

Pipeline (token-major, PE transposes at matmul boundaries, bf16 matmuls):
  gate  = x @ WxT + h_prev @ WshT          -> sigmoid
  sent  = sig(gate) * tanh(cells)
  cv    = V @ WvT  (per batch, 49x49)      -> DRAM scratch, re-read broadcast
  cg    = hiddens @ WgT
  z     = sum_n wh[n] * tanh(cv + cg)      (content tile [tok, 49k, 49n])
  alpha = softmax_k(z)
  cs    = sent @ WsT + cg ; z_ext = wh . tanh(cs)
  beta  = extended-softmax last slot
  c     = alpha @ V  (pair-stacked block-diag matmul)
  out   = (beta*sent + (1-beta)*c + hiddens) @ WmT   (+ bm on host; bm==0 here)
"""

import os
import sys
from contextlib import ExitStack

import numpy as np

sys.path.insert(0, "/opt/trn_rl_repo")

import concourse.bass as bass
import concourse.tile as tile
from concourse import mybir
from concourse.masks import make_identity

F32 = mybir.dt.float32
BF16 = mybir.dt.bfloat16

# per-core dims
BC = 16          # batches per core
T = 64
NTOK = BC * T    # 1024
H = 512
E2 = 1024
KF = 49          # spatial features (and attn dim)
VOC = 10000
P = 128
HK = H // P      # 4
EK = E2 // P     # 8
MT = NTOK // P   # 8 token tiles (2 batches each)
NVT = (VOC + 511) // 512   # 20 vocab n-tiles (last = 272)
VT = (VOC + P - 1) // P    # 79 vocab p-tiles for Wm transpose (last = 16)
OCH = 2560                 # out DMA chunk (4 chunks: 2560*3 + 2320)


def _split_multi_waits(nc):
    """The staged walrus accepts at most ONE embedded sync wait per
    instruction; Tile freely emits several. Hoist the extras onto
    standalone EventSemaphore instructions on the same engine, placed
    immediately before — identical runtime semantics (the engine blocks
    on each in order)."""
    ctr = [0]
    for fn in nc.m.functions:
        for blk in fn.blocks:
            new_insts = []
            for inst in blk.instructions:
                si = inst.sync_info
                waits = list(si.on_wait) if (si is not None and si.on_wait) else []
                if len(waits) > 1:
                    for w in waits[:-1]:
                        ev = mybir.InstEventSemaphore(
                            name=f"EVSPLIT-{ctr[0]}", ins=[], outs=[],
                            sync_info=mybir.SyncInfo(on_wait=[w], on_update=[]),
                        )
                        ev.engine = inst.engine
                        new_insts.append(ev)
                        ctr[0] += 1
                    inst.sync_info = mybir.SyncInfo(
                        on_wait=[waits[-1]],
                        on_update=list(si.on_update) if si.on_update else [],
                    )
                new_insts.append(inst)
            blk.instructions[:] = new_insts
    return nc


def build_core_graph(shift=8.0):
    nc = bass.Bass()

    x_h = nc.declare_dram_parameter("x", [BC, T, E2], F32, isOutput=False)
    hid_h = nc.declare_dram_parameter("hiddens", [BC, T, H], F32, isOutput=False)
    cel_h = nc.declare_dram_parameter("cells", [BC, T, H], F32, isOutput=False)
    v_h = nc.declare_dram_parameter("V", [BC, KF, H], F32, isOutput=False)
    wv_h = nc.declare_dram_parameter("Wv", [KF, H], F32, isOutput=False)
    wg_h = nc.declare_dram_parameter("Wg", [KF, H], F32, isOutput=False)
    ws_h = nc.declare_dram_parameter("Ws", [KF, H], F32, isOutput=False)
    wh_h = nc.declare_dram_parameter("wh", [KF], F32, isOutput=False)
    wx_h = nc.declare_dram_parameter("Wx", [H, E2], F32, isOutput=False)
    wsh_h = nc.declare_dram_parameter("Wsh", [H, H], F32, isOutput=False)
    wm_h = nc.declare_dram_parameter("Wm", [VOC, H], F32, isOutput=False)

    out_h = nc.declare_dram_parameter("out", [BC, T, VOC], F32, isOutput=True)
    alpha_h = nc.declare_dram_parameter("alpha", [BC, T, KF], F32, isOutput=True)
    beta_h = nc.declare_dram_parameter("beta", [BC, T, 1], F32, isOutput=True)

    # internal DRAM scratch for cv: [pair][half][k*n] contiguous bf16
    cv_dram = nc.dram_tensor("cv_scratch", [BC // 2, 2, KF * KF], BF16)
    VOCP = 10112                       # vocab padded to x128 for xbar
    wm_bf_dram = nc.dram_tensor("wm_bf", [VOCP, H], BF16)

    x_flat = x_h[:].flatten_outer_dims()        # [1024, 1024]
    hid_flat = hid_h[:].flatten_outer_dims()    # [1024, 512]
    cel_flat = cel_h[:].flatten_outer_dims()
    v_flat = v_h[:].flatten_outer_dims()        # [784, 512]
    out_flat = out_h[:].flatten_outer_dims()    # [1024, 10000]
    alpha_flat = alpha_h[:].flatten_outer_dims()
    beta_flat = beta_h[:].flatten_outer_dims()

    with tile.TileContext(nc) as tc, ExitStack() as ctx:
        const = ctx.enter_context(tc.tile_pool(name="const", bufs=1))
        # PSUM (8 banks): ptr 2 + pgc 2 + pmain 4
        ptr = ctx.enter_context(tc.tile_pool(name="ptr", bufs=2, space="PSUM"))
        pgc = ctx.enter_context(tc.tile_pool(name="pgc", bufs=2, space="PSUM"))
        pmain = ctx.enter_context(tc.tile_pool(name="pmain", bufs=4, space="PSUM"))
        # setup-only SBUF working pool, released before the main-loop pools
        # open so its zone is reused (stack allocator)
        wctx = ExitStack()
        wpool = wctx.enter_context(tc.tile_pool(name="wpool", bufs=2))
        wpool6 = wctx.enter_context(tc.tile_pool(name="wpool6", bufs=6))

        # ---------------- constants / weights setup ----------------
        id_f32 = const.tile([P, P], F32)
        make_identity(nc, id_f32)
        id_bf = const.tile([P, P], BF16)
        nc.vector.tensor_copy(id_bf, id_f32)

        # tile-0 input loads first so they lead the DMA queues
        pre_x0 = const.tile([P, E2], BF16)
        nc.gpsimd.dma_start(out=pre_x0, in_=x_flat[0:P, :])
        pre_hid0 = const.tile([P, H], F32)
        nc.sync.dma_start(out=pre_hid0, in_=hid_flat[0:P, :])
        pre_cel0 = const.tile([P, H], F32)
        nc.sync.dma_start(out=pre_cel0, in_=cel_flat[0:P, :])

        whrep = const.tile([P, KF], F32)
        wh_ap = wh_h[:]
        nc.gpsimd.dma_start(
            out=whrep,
            in_=bass.AP(tensor=wh_ap.tensor, offset=wh_ap.offset,
                        ap=[[0, P], [1, KF]]),
        )
        whrep_bf = const.tile([P, KF], BF16)
        nc.vector.tensor_copy(whrep_bf, whrep)
        negshift = const.tile([P, 1], F32)
        nc.vector.memset(negshift, -shift)

        def transpose_weight(dst, src_h, rows):
            # src [rows<=128, 512] f32 DRAM -> dst [128, 4, rows] BF16
            w_sb = wpool.tile([rows, H], F32, tag="w_sb")
            nc.sync.dma_start(out=w_sb, in_=src_h)
            ps = ptr.tile([P, 512], F32, tag="ptr")
            for kt in range(HK):
                nc.tensor.transpose(
                    ps[:, kt * P:kt * P + rows],
                    w_sb[:, kt * P:(kt + 1) * P],
                    id_f32[:rows, :rows],
                )
            nc.vector.tensor_copy(
                dst[:, :, :],
                ps.rearrange("p (k r) -> p k r", k=HK)[:, :, :rows],
            )

        # WxT [128, 8, 512] : e on partitions (8 e-tiles), h on free
        WxT = const.tile([P, EK, H], BF16)
        for ht in range(HK):
            wx_sb = wpool.tile([P, E2], F32, tag="wx_sb")
            nc.sync.dma_start(out=wx_sb, in_=wx_h[ht * P:(ht + 1) * P, :])
            for eg in range(2):          # two psum banks of 4 transposes
                ps = ptr.tile([P, 512], F32, tag="ptr")
                for j in range(4):
                    et = eg * 4 + j
                    nc.tensor.transpose(
                        ps[:, j * P:(j + 1) * P],
                        wx_sb[:, et * P:(et + 1) * P], id_f32)
                nc.vector.tensor_copy(
                    WxT[:, eg * 4:(eg + 1) * 4, ht * P:(ht + 1) * P], ps)

        # WshT [128, 4, 512]
        WshT = const.tile([P, HK, H], BF16)
        for ht in range(HK):
            wsh_sb = wpool.tile([P, H], F32, tag="wsh_sb")
            nc.sync.dma_start(out=wsh_sb, in_=wsh_h[ht * P:(ht + 1) * P, :])
            ps = ptr.tile([P, 512], F32, tag="ptr")
            for it in range(HK):
                nc.tensor.transpose(
                    ps[:, it * P:(it + 1) * P],
                    wsh_sb[:, it * P:(it + 1) * P], id_f32)
            nc.vector.tensor_copy(WshT[:, :, ht * P:(ht + 1) * P], ps)

        # WvT / WgT / WsT [128, 4, 49]
        WvT = const.tile([P, HK, KF], BF16)
        WgT = const.tile([P, HK, KF], BF16)
        WsT = const.tile([P, HK, KF], BF16)
        transpose_weight(WvT, wv_h[:, :], KF)
        transpose_weight(WgT, wg_h[:, :], KF)
        transpose_weight(WsT, ws_h[:, :], KF)

        # V2 [98, 8, 512] bf16 pair-stacked (cast during DMA)
        V2 = const.tile([2 * KF, BC // 2, H], BF16)
        for pr in range(BC // 2):
            nc.gpsimd.dma_start(
                out=V2[:, pr, :],
                in_=v_flat[pr * 2 * KF:(pr + 1) * 2 * KF, :],
            )

        # cv per pair: transpose V pairs, matmul with WvT, stash to DRAM
        cv_all = const.tile([2 * KF, BC // 2, KF], BF16)
        for pr in range(BC // 2):
            v_sb = wpool.tile([2 * KF, H], F32, tag="v_sb")
            nc.sync.dma_start(
                out=v_sb, in_=v_flat[pr * 2 * KF:(pr + 1) * 2 * KF, :])
            vT = wpool.tile([P, HK, 2 * KF], BF16, tag="vT")
            ps = ptr.tile([P, 512], F32, tag="ptr")
            for kt in range(HK):
                nc.tensor.transpose(
                    ps[:, kt * P:kt * P + 2 * KF],
                    v_sb[:, kt * P:(kt + 1) * P],
                    id_f32[:2 * KF, :2 * KF],
                )
            nc.vector.tensor_copy(
                vT[:, :, :],
                ps.rearrange("p (k r) -> p k r", k=HK)[:, :, :2 * KF],
            )
            pcv = pgc.tile([2 * KF, 512], F32, tag="pgc")
            for kt in range(HK):
                nc.tensor.matmul(
                    pcv[:, :KF], vT[:, kt, :], WvT[:, kt, :],
                    start=(kt == 0), stop=(kt == HK - 1),
                )
            nc.vector.tensor_copy(cv_all[:, pr, :], pcv[:, :KF])
        for pr in range(BC // 2):
            nc.sync.dma_start(
                out=bass.AP(tensor=cv_dram, offset=pr * 2 * KF * KF,
                            ap=[[KF * KF, 2], [KF, KF], [1, KF]]),
                in_=cv_all[:, pr, :],
            )

        # WmT [128, 4, 10000] bf16: cast-load Wm tiles, PE-transpose (bf16)
        WmT = const.tile([P, HK, VOC], BF16)
        for vt in range(VT):
            pv = min(P, VOC - vt * P)
            wm_sb = wpool6.tile([P, H], BF16, tag="wm_sb")
            nc.gpsimd.dma_start(
                out=wm_sb[:pv, :], in_=wm_h[vt * P:vt * P + pv, :])
            ps = ptr.tile([P, 512], BF16, tag="ptr")
            for kt in range(HK):
                nc.tensor.transpose(
                    ps[:, kt * P:kt * P + pv],
                    wm_sb[:pv, kt * P:(kt + 1) * P],
                    id_bf[:pv, :pv],
                )
            nc.vector.tensor_copy(
                WmT[:, :, vt * P:vt * P + pv],
                ps.rearrange("p (k r) -> p k r", k=HK)[:, :, :pv],
            )

        # release setup pool, open main-loop pools in its place
        wctx.close()
        actp = ctx.enter_context(tc.tile_pool(name="actp", bufs=3))
        work = ctx.enter_context(tc.tile_pool(name="work", bufs=2))
        trp = ctx.enter_context(tc.tile_pool(name="trp", bufs=3))
        contp = ctx.enter_context(tc.tile_pool(name="contp", bufs=2))
        smallp = ctx.enter_context(tc.tile_pool(name="smallp", bufs=3))
        outp = ctx.enter_context(tc.tile_pool(name="outp", bufs=4))

        # ---------------- main loop over token tiles ----------------
        # software-pipelined: tile m+1's "head" (loads, hT, cg, content
        # chain) is issued before tile m's main-projection burst, so its
        # transposes/psums are in flight while PE crunches the mains.

        def produce_head(m):
            R0 = m * P
            st = {}
            content = contp.tile([P, KF, KF], BF16, tag="content")
            for half in range(2):
                nc.gpsimd.dma_start(
                    out=content[half * T:(half + 1) * T, :, :],
                    in_=bass.AP(
                        tensor=cv_dram,
                        offset=(m * 2 + half) * KF * KF,
                        ap=[[0, T], [KF, KF], [1, KF]],
                    ),
                )
            if m == 0:
                x_bf, hid, cel = pre_x0, pre_hid0, pre_cel0
            else:
                x_bf = actp.tile([P, E2], BF16, tag="x_bf")
                nc.gpsimd.dma_start(out=x_bf, in_=x_flat[R0:R0 + P, :])
                hid = actp.tile([P, H], F32, tag="hid")
                nc.sync.dma_start(out=hid, in_=hid_flat[R0:R0 + P, :])
                cel = actp.tile([P, H], F32, tag="cel")
                nc.sync.dma_start(out=cel, in_=cel_flat[R0:R0 + P, :])

            h_bf = work.tile([P, H], BF16, tag="h_bf")
            nc.vector.tensor_copy(h_bf, hid)
            hT = trp.tile([P, HK, P], BF16, tag="hT")
            ps_h = ptr.tile([P, 512], BF16, tag="ptr")
            for kt in range(HK):
                nc.tensor.transpose(
                    ps_h[:, kt * P:(kt + 1) * P],
                    h_bf[:, kt * P:(kt + 1) * P], id_bf)
            nc.vector.tensor_copy(hT[:, :, :], ps_h)

            xT = trp.tile([P, EK, P], BF16, tag="xT")
            for eg in range(2):
                ps = ptr.tile([P, 512], BF16, tag="ptr")
                for j in range(4):
                    et = eg * 4 + j
                    nc.tensor.transpose(
                        ps[:, j * P:(j + 1) * P],
                        x_bf[:, et * P:(et + 1) * P], id_bf)
                nc.vector.tensor_copy(xT[:, eg * 4:(eg + 1) * 4, :], ps)

            hpT = trp.tile([P, HK, P], BF16, tag="hpT")
            for kt in range(HK):
                nc.vector.memset(hpT[:, kt, 0:1], 0.0)
                nc.vector.memset(hpT[:, kt, T:T + 1], 0.0)
                nc.vector.tensor_copy(hpT[:, kt, 1:T], hT[:, kt, 0:T - 1])
                nc.vector.tensor_copy(hpT[:, kt, T + 1:P], hT[:, kt, T:P - 1])

            # cg = hiddens@WgT  -> content chain -> zfull[:, :49]
            pcg = pgc.tile([P, 512], F32, tag="pgc")
            for kt in range(HK):
                nc.tensor.matmul(pcg[:, :KF], hT[:, kt, :], WgT[:, kt, :],
                                 start=(kt == 0), stop=(kt == HK - 1))
            cg = smallp.tile([P, KF], F32, tag="cg")
            nc.vector.tensor_copy(cg, pcg[:, :KF])
            cg_bf = smallp.tile([P, KF], BF16, tag="cg_bf")
            nc.vector.tensor_copy(cg_bf, pcg[:, :KF])

            zfull = smallp.tile([P, KF + 1], F32, tag="zfull")
            cg_b = cg_bf.unsqueeze(1).to_broadcast((P, KF, KF))
            nc.vector.tensor_tensor(content, content, cg_b,
                                    op=mybir.AluOpType.add)
            nc.scalar.activation(content, content,
                                 mybir.ActivationFunctionType.Tanh)
            wh_b = whrep_bf.unsqueeze(1).to_broadcast((P, KF, KF))
            nc.vector.tensor_tensor(content, content, wh_b,
                                    op=mybir.AluOpType.mult)
            nc.vector.reduce_sum(zfull[:, 0:KF], content,
                                 axis=mybir.AxisListType.X)

            st.update(hid=hid, cel=cel, hT=hT, xT=xT, hpT=hpT,
                      cg=cg, zfull=zfull)
            return st

        head = produce_head(0)
        for m in range(MT):
            R0 = m * P
            hid = head["hid"]; cel = head["cel"]; hT = head["hT"]
            xT = head["xT"]; hpT = head["hpT"]; cg = head["cg"]
            zfull = head["zfull"]

            pgate = pgc.tile([P, 512], F32, tag="pgc")
            nmm = EK + HK
            i = 0
            for et in range(EK):
                nc.tensor.matmul(pgate, xT[:, et, :], WxT[:, et, :],
                                 start=(i == 0), stop=(i == nmm - 1))
                i += 1
            for kt in range(HK):
                nc.tensor.matmul(pgate, hpT[:, kt, :], WshT[:, kt, :],
                                 start=(i == 0), stop=(i == nmm - 1))
                i += 1

            sig = work.tile([P, H], F32, tag="sig")
            nc.scalar.activation(sig, pgate, mybir.ActivationFunctionType.Sigmoid)
            tnc = work.tile([P, H], F32, tag="tnc")
            nc.scalar.activation(tnc, cel, mybir.ActivationFunctionType.Tanh)
            sent = work.tile([P, H], F32, tag="sent")
            nc.vector.tensor_mul(sent, sig, tnc)

            # cs = sent@WsT + cg ; z_ext = wh . tanh(cs)
            sent_bf = work.tile([P, H], BF16, tag="sent_bf")
            nc.vector.tensor_copy(sent_bf, sent)
            sT = trp.tile([P, HK, P], BF16, tag="sT")
            ps_s = ptr.tile([P, 512], BF16, tag="ptr")
            for kt in range(HK):
                nc.tensor.transpose(
                    ps_s[:, kt * P:(kt + 1) * P],
                    sent_bf[:, kt * P:(kt + 1) * P], id_bf)
            nc.vector.tensor_copy(sT[:, :, :], ps_s)
            pcs = pgc.tile([P, 512], F32, tag="pgc")
            for kt in range(HK):
                nc.tensor.matmul(pcs[:, :KF], sT[:, kt, :], WsT[:, kt, :],
                                 start=(kt == 0), stop=(kt == HK - 1))
            cs = smallp.tile([P, KF], F32, tag="cs")
            nc.vector.tensor_add(cs, pcs[:, :KF], cg)
            tcs = smallp.tile([P, KF], F32, tag="tcs")
            nc.scalar.activation(tcs, cs, mybir.ActivationFunctionType.Tanh)
            ttmp = smallp.tile([P, KF], F32, tag="ttmp")
            nc.vector.tensor_mul(ttmp, tcs, whrep)
            nc.vector.reduce_sum(zfull[:, KF:KF + 1], ttmp,
                                 axis=mybir.AxisListType.X)

            # issue next tile's head here: PE fills the softmax/c/chat
            # serial section with next-tile transposes + cg matmuls
            if m + 1 < MT:
                head = produce_head(m + 1)

            # fixed-shift exp (shift = sum|wh|+1 bound, baked at build):
            # e49 feeds the c-matmul without waiting on the sentinel chain
            e49 = smallp.tile([P, KF], F32, tag="e49")
            s1 = smallp.tile([P, 1], F32, tag="s1")
            nc.scalar.activation(e49, zfull[:, 0:KF],
                                 mybir.ActivationFunctionType.Exp,
                                 bias=negshift, accum_out=s1)
            rs1 = smallp.tile([P, 1], F32, tag="rs1")
            nc.vector.reciprocal(rs1, s1)
            alpha_sb = smallp.tile([P, KF], F32, tag="alpha_sb")
            nc.vector.tensor_scalar_mul(alpha_sb, e49, rs1)
            nc.sync.dma_start(out=alpha_flat[R0:R0 + P, :], in_=alpha_sb)

            e_ext = smallp.tile([P, 1], F32, tag="e_ext")
            nc.scalar.activation(e_ext, zfull[:, KF:KF + 1],
                                 mybir.ActivationFunctionType.Exp,
                                 bias=negshift)
            den = smallp.tile([P, 1], F32, tag="den")
            nc.vector.tensor_add(den, s1, e_ext)
            rden = smallp.tile([P, 1], F32, tag="rden")
            nc.vector.reciprocal(rden, den)
            beta_sb = smallp.tile([P, 1], F32, tag="beta_sb")
            nc.vector.tensor_mul(beta_sb, e_ext, rden)
            nc.sync.dma_start(out=beta_flat[R0:R0 + P, :], in_=beta_sb)

            # c*s1 = e49 @ V  via block-diag pair trick
            a2 = smallp.tile([P, 2 * KF], F32, tag="a2")
            nc.vector.memset(a2, 0.0)
            nc.vector.tensor_copy(a2[0:T, 0:KF], e49[0:T, :])
            nc.vector.tensor_copy(a2[T:P, KF:2 * KF], e49[T:P, :])
            psA = ptr.tile([P, 512], F32, tag="ptr")
            nc.tensor.transpose(psA[:2 * KF, :P], a2, id_f32)
            a2T = trp.tile([2 * KF, P], BF16, tag="a2T")
            nc.vector.tensor_copy(a2T, psA[:2 * KF, :P])
            pc = pgc.tile([P, 512], F32, tag="pgc")
            nc.tensor.matmul(pc, a2T, V2[:, m, :], start=True, stop=True)

            # a = (1-beta)*(pc*rs1) + beta*sent + hid, fused:
            # rob = rs1*(1-beta); t1 = pc*rob + hid; a = sent*beta + t1
            omb = smallp.tile([P, 1], F32, tag="omb")
            nc.vector.tensor_scalar(out=omb, in0=beta_sb, scalar1=-1.0,
                                    scalar2=1.0, op0=mybir.AluOpType.mult,
                                    op1=mybir.AluOpType.add)
            rob = smallp.tile([P, 1], F32, tag="rob")
            nc.vector.tensor_mul(rob, rs1, omb)
            t1 = work.tile([P, H], F32, tag="t1")
            nc.vector.scalar_tensor_tensor(
                out=t1, in0=pc, scalar=rob, in1=hid,
                op0=mybir.AluOpType.mult, op1=mybir.AluOpType.add,
            )
            a_bf = work.tile([P, H], BF16, tag="a_bf")
            nc.vector.scalar_tensor_tensor(
                out=a_bf, in0=sent, scalar=beta_sb, in1=t1,
                op0=mybir.AluOpType.mult, op1=mybir.AluOpType.add,
            )
            aT = trp.tile([P, HK, P], BF16, tag="aT")
            ps_a = ptr.tile([P, 512], BF16, tag="ptr")
            for kt in range(HK):
                nc.tensor.transpose(
                    ps_a[:, kt * P:(kt + 1) * P],
                    a_bf[:, kt * P:(kt + 1) * P], id_bf)
            nc.vector.tensor_copy(aT[:, :, :], ps_a)

            # out = a @ WmT ; evacs alternate DVE/ACT, 7 mid-size DMAs
            for oc in range(7):
                base = oc * 1536
                width = min(1536, VOC - base)
                osb = outp.tile([P, 1536], F32, tag="osb")
                nsub = (width + 511) // 512
                for s in range(nsub):
                    nb = base + s * 512
                    w = min(512, VOC - nb)
                    po = pmain.tile([P, 512], F32, tag="pmain")
                    for kt in range(HK):
                        nc.tensor.matmul(
                            po[:, :w], aT[:, kt, :],
                            WmT[:, kt, nb:nb + w],
                            start=(kt == 0), stop=(kt == HK - 1),
                        )
                    if s % 2 == 0:
                        nc.vector.tensor_copy(osb[:, s * 512:s * 512 + w],
                                              po[:, :w])
                    else:
                        nc.scalar.copy(osb[:, s * 512:s * 512 + w], po[:, :w])
                nc.sync.dma_start(
                    out=out_flat[R0:R0 + P, base:base + width],
                    in_=osb[:, :width],
                )

    _split_multi_waits(nc)
    return nc


_GRAPH_CACHE = {}


def _get_graph(shift):
    key = round(float(shift), 3)
    if key not in _GRAPH_CACHE:
        _GRAPH_CACHE[key] = build_core_graph(key)
    return _GRAPH_CACHE[key]


def kernel(x, hiddens, cells, V, Wv, Wg, Ws, wh, Wx, Wsh, Wm, bm):
    from concourse.bass_utils import run_bass_kernel_spmd

    n_cores = 8
    B = x.shape[0]
    bc = B // n_cores

    weights = {
        "Wv": np.ascontiguousarray(Wv, np.float32),
        "Wg": np.ascontiguousarray(Wg, np.float32),
        "Ws": np.ascontiguousarray(Ws, np.float32),
        "wh": np.ascontiguousarray(wh, np.float32),
        "Wx": np.ascontiguousarray(Wx, np.float32),
        "Wsh": np.ascontiguousarray(Wsh, np.float32),
        "Wm": np.ascontiguousarray(Wm, np.float32),
    }
    in_maps = []
    for i in range(n_cores):
        sl = slice(i * bc, (i + 1) * bc)
        m = {
            "x": np.ascontiguousarray(x[sl], np.float32),
            "hiddens": np.ascontiguousarray(hiddens[sl], np.float32),
            "cells": np.ascontiguousarray(cells[sl], np.float32),
            "V": np.ascontiguousarray(V[sl], np.float32),
        }
        m.update(weights)
        in_maps.append(m)

    shift = float(np.abs(np.asarray(wh, np.float64)).sum()) + 1.0
    nc = _get_graph(shift)
    trace = bool(int(os.environ.get("KERNEL_TRACE", "0")))
    res = run_bass_kernel_spmd(nc, in_maps, core_ids=list(range(n_cores)),
                               trace=trace)
    if trace:
        kernel.last_exec_time_ns = res.exec_time_ns
        kernel.last_profile = res

    out = np.concatenate([r["out"] for r in res.results], axis=0)
    alpha = np.concatenate([r["alpha"] for r in res.results], axis=0)
    beta = np.concatenate([r["beta"] for r in res.results], axis=0)
    if np.any(bm):
        out = out + np.asarray(bm, np.float32)
    return out, alpha, beta


# revision 28
# speedup vs baseline: 1.2364x; 1.0013x over previous
"""Trainium2 Bass kernel for the Adaptive attention-sentinel module.

Full inputs -> data-parallel over batch B=128 across 8 NeuronCores
(16 batches/core). Each core runs an identical SPMD program on its
batch shard; outputs are concatenated on the host.

Per-core shapes (hardcoded):
  x       [16, 64, 1024]   tokens = 16*64 = 1024
  hiddens [16, 64, 512]
  cells   [16, 64, 512]
  V       [16, 49, 512]
  weights full (replicated): Wv/Wg/Ws [49,512], wh [49],
  Wx [512,1024], Wsh [512,512], Wm [10000,512], bm [10000]

Pipeline (token-major, PE transposes at matmul boundaries, bf16 matmuls):
  gate  = x @ WxT + h_prev @ WshT          -> sigmoid
  sent  = sig(gate) * tanh(cells)
  cv    = V @ WvT  (per batch, 49x49)      -> DRAM scratch, re-read broadcast
  cg    = hiddens @ WgT
  z     = sum_n wh[n] * tanh(cv + cg)      (content tile [tok, 49k, 49n])
  alpha = softmax_k(z)
  cs    = sent @ WsT + cg ; z_ext = wh . tanh(cs)
  beta  = extended-softmax last slot
  c     = alpha @ V  (pair-stacked block-diag matmul)
  out   = (beta*sent + (1-beta)*c + hiddens) @ WmT   (+ bm on host; bm==0 here)
"""

import os
import sys
from contextlib import ExitStack

import numpy as np

sys.path.insert(0, "/opt/trn_rl_repo")

import concourse.bass as bass
import concourse.tile as tile
from concourse import mybir
from concourse.masks import make_identity

F32 = mybir.dt.float32
BF16 = mybir.dt.bfloat16

# per-core dims
BC = 16          # batches per core
T = 64
NTOK = BC * T    # 1024
H = 512
E2 = 1024
KF = 49          # spatial features (and attn dim)
VOC = 10000
P = 128
HK = H // P      # 4
EK = E2 // P     # 8
MT = NTOK // P   # 8 token tiles (2 batches each)
NVT = (VOC + 511) // 512   # 20 vocab n-tiles (last = 272)
VT = (VOC + P - 1) // P    # 79 vocab p-tiles for Wm transpose (last = 16)
OCH = 2560                 # out DMA chunk (4 chunks: 2560*3 + 2320)


def _split_multi_waits(nc):
    """The staged walrus accepts at most ONE embedded sync wait per
    instruction; Tile freely emits several. Hoist the extras onto
    standalone EventSemaphore instructions on the same engine, placed
    immediately before — identical runtime semantics (the engine blocks
    on each in order)."""
    ctr = [0]
    for fn in nc.m.functions:
        for blk in fn.blocks:
            new_insts = []
            for inst in blk.instructions:
                si = inst.sync_info
                waits = list(si.on_wait) if (si is not None and si.on_wait) else []
                if len(waits) > 1:
                    for w in waits[:-1]:
                        ev = mybir.InstEventSemaphore(
                            name=f"EVSPLIT-{ctr[0]}", ins=[], outs=[],
                            sync_info=mybir.SyncInfo(on_wait=[w], on_update=[]),
                        )
                        ev.engine = inst.engine
                        new_insts.append(ev)
                        ctr[0] += 1
                    inst.sync_info = mybir.SyncInfo(
                        on_wait=[waits[-1]],
                        on_update=list(si.on_update) if si.on_update else [],
                    )
                new_insts.append(inst)
            blk.instructions[:] = new_insts
    return nc


def build_core_graph(shift=8.0):
    nc = bass.Bass()

    x_h = nc.declare_dram_parameter("x", [BC, T, E2], F32, isOutput=False)
    hid_h = nc.declare_dram_parameter("hiddens", [BC, T, H], F32, isOutput=False)
    cel_h = nc.declare_dram_parameter("cells", [BC, T, H], F32, isOutput=False)
    v_h = nc.declare_dram_parameter("V", [BC, KF, H], F32, isOutput=False)
    wv_h = nc.declare_dram_parameter("Wv", [KF, H], F32, isOutput=False)
    wg_h = nc.declare_dram_parameter("Wg", [KF, H], F32, isOutput=False)
    ws_h = nc.declare_dram_parameter("Ws", [KF, H], F32, isOutput=False)
    wh_h = nc.declare_dram_parameter("wh", [KF], F32, isOutput=False)
    wx_h = nc.declare_dram_parameter("Wx", [H, E2], F32, isOutput=False)
    wsh_h = nc.declare_dram_parameter("Wsh", [H, H], F32, isOutput=False)
    wm_h = nc.declare_dram_parameter("Wm", [VOC, H], F32, isOutput=False)

    out_h = nc.declare_dram_parameter("out", [BC, T, VOC], F32, isOutput=True)
    alpha_h = nc.declare_dram_parameter("alpha", [BC, T, KF], F32, isOutput=True)
    beta_h = nc.declare_dram_parameter("beta", [BC, T, 1], F32, isOutput=True)

    # internal DRAM scratch for cv: [pair][half][k*n] contiguous bf16
    cv_dram = nc.dram_tensor("cv_scratch", [BC // 2, 2, KF * KF], BF16)
    VOCP = 10112                       # vocab padded to x128 for xbar
    wm_bf_dram = nc.dram_tensor("wm_bf", [VOCP, H], BF16)

    x_flat = x_h[:].flatten_outer_dims()        # [1024, 1024]
    hid_flat = hid_h[:].flatten_outer_dims()    # [1024, 512]
    cel_flat = cel_h[:].flatten_outer_dims()
    v_flat = v_h[:].flatten_outer_dims()        # [784, 512]
    out_flat = out_h[:].flatten_outer_dims()    # [1024, 10000]
    alpha_flat = alpha_h[:].flatten_outer_dims()
    beta_flat = beta_h[:].flatten_outer_dims()

    with tile.TileContext(nc) as tc, ExitStack() as ctx:
        const = ctx.enter_context(tc.tile_pool(name="const", bufs=1))
        # PSUM (8 banks): ptr 2 + pgc 2 + pmain 4
        ptr = ctx.enter_context(tc.tile_pool(name="ptr", bufs=2, space="PSUM"))
        pgc = ctx.enter_context(tc.tile_pool(name="pgc", bufs=2, space="PSUM"))
        pmain = ctx.enter_context(tc.tile_pool(name="pmain", bufs=4, space="PSUM"))
        # setup-only SBUF working pool, released before the main-loop pools
        # open so its zone is reused (stack allocator)
        wctx = ExitStack()
        wpool = wctx.enter_context(tc.tile_pool(name="wpool", bufs=2))
        wpool6 = wctx.enter_context(tc.tile_pool(name="wpool6", bufs=2))

        # ---------------- constants / weights setup ----------------
        id_f32 = const.tile([P, P], F32)
        make_identity(nc, id_f32)
        id_bf = const.tile([P, P], BF16)
        nc.vector.tensor_copy(id_bf, id_f32)

        # tile-0 input loads first so they lead the DMA queues
        pre_x0 = const.tile([P, E2], BF16)
        nc.gpsimd.dma_start(out=pre_x0, in_=x_flat[0:P, :])
        pre_hid0 = const.tile([P, H], F32)
        nc.sync.dma_start(out=pre_hid0, in_=hid_flat[0:P, :])
        pre_cel0 = const.tile([P, H], F32)
        nc.sync.dma_start(out=pre_cel0, in_=cel_flat[0:P, :])

        whrep = const.tile([P, KF], F32)
        wh_ap = wh_h[:]
        nc.gpsimd.dma_start(
            out=whrep,
            in_=bass.AP(tensor=wh_ap.tensor, offset=wh_ap.offset,
                        ap=[[0, P], [1, KF]]),
        )
        whrep_bf = const.tile([P, KF], BF16)
        nc.vector.tensor_copy(whrep_bf, whrep)
        negshift = const.tile([P, 1], F32)
        nc.vector.memset(negshift, -shift)

        def transpose_weight(dst, src_h, rows):
            # src [rows<=128, 512] f32 DRAM -> dst [128, 4, rows] BF16
            w_sb = wpool.tile([rows, H], F32, tag="w_sb")
            nc.sync.dma_start(out=w_sb, in_=src_h)
            ps = ptr.tile([P, 512], F32, tag="ptr")
            for kt in range(HK):
                nc.tensor.transpose(
                    ps[:, kt * P:kt * P + rows],
                    w_sb[:, kt * P:(kt + 1) * P],
                    id_f32[:rows, :rows],
                )
            nc.vector.tensor_copy(
                dst[:, :, :],
                ps.rearrange("p (k r) -> p k r", k=HK)[:, :, :rows],
            )

        # WxT [128, 8, 512] : e on partitions (8 e-tiles), h on free
        WxT = const.tile([P, EK, H], BF16)
        for ht in range(HK):
            wx_sb = wpool.tile([P, E2], F32, tag="wx_sb")
            nc.sync.dma_start(out=wx_sb, in_=wx_h[ht * P:(ht + 1) * P, :])
            for eg in range(2):          # two psum banks of 4 transposes
                ps = ptr.tile([P, 512], F32, tag="ptr")
                for j in range(4):
                    et = eg * 4 + j
                    nc.tensor.transpose(
                        ps[:, j * P:(j + 1) * P],
                        wx_sb[:, et * P:(et + 1) * P], id_f32)
                nc.vector.tensor_copy(
                    WxT[:, eg * 4:(eg + 1) * 4, ht * P:(ht + 1) * P], ps)

        # WshT [128, 4, 512]
        WshT = const.tile([P, HK, H], BF16)
        for ht in range(HK):
            wsh_sb = wpool.tile([P, H], F32, tag="wsh_sb")
            nc.sync.dma_start(out=wsh_sb, in_=wsh_h[ht * P:(ht + 1) * P, :])
            ps = ptr.tile([P, 512], F32, tag="ptr")
            for it in range(HK):
                nc.tensor.transpose(
                    ps[:, it * P:(it + 1) * P],
                    wsh_sb[:, it * P:(it + 1) * P], id_f32)
            nc.vector.tensor_copy(WshT[:, :, ht * P:(ht + 1) * P], ps)

        # WvT / WgT / WsT [128, 4, 49]
        WvT = const.tile([P, HK, KF], BF16)
        WgT = const.tile([P, HK, KF], BF16)
        WsT = const.tile([P, HK, KF], BF16)
        transpose_weight(WvT, wv_h[:, :], KF)
        transpose_weight(WgT, wg_h[:, :], KF)
        transpose_weight(WsT, ws_h[:, :], KF)

        # V2 [98, 8, 512] bf16 pair-stacked (cast during DMA)
        V2 = const.tile([2 * KF, BC // 2, H], BF16)
        for pr in range(BC // 2):
            nc.gpsimd.dma_start(
                out=V2[:, pr, :],
                in_=v_flat[pr * 2 * KF:(pr + 1) * 2 * KF, :],
            )

        # cv per pair: transpose V pairs, matmul with WvT, stash to DRAM
        cv_all = const.tile([2 * KF, BC // 2, KF], BF16)
        for pr in range(BC // 2):
            v_sb = wpool.tile([2 * KF, H], F32, tag="v_sb")
            nc.sync.dma_start(
                out=v_sb, in_=v_flat[pr * 2 * KF:(pr + 1) * 2 * KF, :])
            vT = wpool.tile([P, HK, 2 * KF], BF16, tag="vT")
            ps = ptr.tile([P, 512], F32, tag="ptr")
            for kt in range(HK):
                nc.tensor.transpose(
                    ps[:, kt * P:kt * P + 2 * KF],
                    v_sb[:, kt * P:(kt + 1) * P],
                    id_f32[:2 * KF, :2 * KF],
                )
            nc.vector.tensor_copy(
                vT[:, :, :],
                ps.rearrange("p (k r) -> p k r", k=HK)[:, :, :2 * KF],
            )
            pcv = pgc.tile([2 * KF, 512], F32, tag="pgc")
            for kt in range(HK):
                nc.tensor.matmul(
                    pcv[:, :KF], vT[:, kt, :], WvT[:, kt, :],
                    start=(kt == 0), stop=(kt == HK - 1),
                )
            nc.vector.tensor_copy(cv_all[:, pr, :], pcv[:, :KF])
        for pr in range(BC // 2):
            nc.sync.dma_start(
                out=bass.AP(tensor=cv_dram, offset=pr * 2 * KF * KF,
                            ap=[[KF * KF, 2], [KF, KF], [1, KF]]),
                in_=cv_all[:, pr, :],
            )

        # WmT [128, 4, 10000] bf16: batched cast-loads (8 v-tiles per
        # SWDGE dispatch keeps the Q7 queue short), PE-transpose (bf16)
        WmT = const.tile([P, HK, VOC], BF16)
        wm_ap = wm_h[:, :]
        chunks = [(c * 8, 8) for c in range(9)] + [(72, 6), (78, 1)]
        for vt0, nv in chunks:
            pv_last = min(P, VOC - (vt0 + nv - 1) * P)
            full = nv - (1 if pv_last < P else 0)
            wm_sb = wpool6.tile([P, 8, H], BF16, tag="wm_sb")
            if full > 0:
                nc.gpsimd.dma_start(
                    out=wm_sb[:, :full, :],
                    in_=bass.AP(tensor=wm_ap.tensor,
                                offset=vt0 * P * H,
                                ap=[[H, P], [P * H, full], [1, H]]),
                )
            if pv_last < P:
                nc.gpsimd.dma_start(
                    out=wm_sb[:pv_last, full, :],
                    in_=wm_h[(vt0 + full) * P:(vt0 + full) * P + pv_last, :],
                )
            for j in range(nv):
                vt = vt0 + j
                pv = min(P, VOC - vt * P)
                ps = ptr.tile([P, 512], BF16, tag="ptr")
                for kt in range(HK):
                    nc.tensor.transpose(
                        ps[:, kt * P:kt * P + pv],
                        wm_sb[:pv, j, kt * P:(kt + 1) * P],
                        id_bf[:pv, :pv],
                    )
                nc.vector.tensor_copy(
                    WmT[:, :, vt * P:vt * P + pv],
                    ps.rearrange("p (k r) -> p k r", k=HK)[:, :, :pv],
                )

        # release setup pool, open main-loop pools in its place
        wctx.close()
        actp = ctx.enter_context(tc.tile_pool(name="actp", bufs=3))
        work = ctx.enter_context(tc.tile_pool(name="work", bufs=2))
        trp = ctx.enter_context(tc.tile_pool(name="trp", bufs=3))
        contp = ctx.enter_context(tc.tile_pool(name="contp", bufs=2))
        smallp = ctx.enter_context(tc.tile_pool(name="smallp", bufs=3))
        outp = ctx.enter_context(tc.tile_pool(name="outp", bufs=4))

        # ---------------- main loop over token tiles ----------------
        # software-pipelined: tile m+1's "head" (loads, hT, cg, content
        # chain) is issued before tile m's main-projection burst, so its
        # transposes/psums are in flight while PE crunches the mains.

        def produce_head(m):
            R0 = m * P
            st = {}
            content = contp.tile([P, KF, KF], BF16, tag="content")
            for half in range(2):
                nc.gpsimd.dma_start(
                    out=content[half * T:(half + 1) * T, :, :],
                    in_=bass.AP(
                        tensor=cv_dram,
                        offset=(m * 2 + half) * KF * KF,
                        ap=[[0, T], [KF, KF], [1, KF]],
                    ),
                )
            if m == 0:
                x_bf, hid, cel = pre_x0, pre_hid0, pre_cel0
            else:
                x_bf = actp.tile([P, E2], BF16, tag="x_bf")
                nc.gpsimd.dma_start(out=x_bf, in_=x_flat[R0:R0 + P, :])
                hid = actp.tile([P, H], F32, tag="hid")
                nc.sync.dma_start(out=hid, in_=hid_flat[R0:R0 + P, :])
                cel = actp.tile([P, H], F32, tag="cel")
                nc.sync.dma_start(out=cel, in_=cel_flat[R0:R0 + P, :])

            h_bf = work.tile([P, H], BF16, tag="h_bf")
            nc.vector.tensor_copy(h_bf, hid)
            hT = trp.tile([P, HK, P], BF16, tag="hT")
            ps_h = ptr.tile([P, 512], BF16, tag="ptr")
            for kt in range(HK):
                nc.tensor.transpose(
                    ps_h[:, kt * P:(kt + 1) * P],
                    h_bf[:, kt * P:(kt + 1) * P], id_bf)
            nc.vector.tensor_copy(hT[:, :, :], ps_h)

            xT = trp.tile([P, EK, P], BF16, tag="xT")
            for eg in range(2):
                ps = ptr.tile([P, 512], BF16, tag="ptr")
                for j in range(4):
                    et = eg * 4 + j
                    nc.tensor.transpose(
                        ps[:, j * P:(j + 1) * P],
                        x_bf[:, et * P:(et + 1) * P], id_bf)
                nc.vector.tensor_copy(xT[:, eg * 4:(eg + 1) * 4, :], ps)

            hpT = trp.tile([P, HK, P], BF16, tag="hpT")
            for kt in range(HK):
                nc.vector.memset(hpT[:, kt, 0:1], 0.0)
                nc.vector.memset(hpT[:, kt, T:T + 1], 0.0)
                nc.vector.tensor_copy(hpT[:, kt, 1:T], hT[:, kt, 0:T - 1])
                nc.vector.tensor_copy(hpT[:, kt, T + 1:P], hT[:, kt, T:P - 1])

            # cg = hiddens@WgT  -> content chain -> zfull[:, :49]
            pcg = pgc.tile([P, 512], F32, tag="pgc")
            for kt in range(HK):
                nc.tensor.matmul(pcg[:, :KF], hT[:, kt, :], WgT[:, kt, :],
                                 start=(kt == 0), stop=(kt == HK - 1))
            cg = smallp.tile([P, KF], F32, tag="cg")
            nc.vector.tensor_copy(cg, pcg[:, :KF])
            cg_bf = smallp.tile([P, KF], BF16, tag="cg_bf")
            nc.vector.tensor_copy(cg_bf, pcg[:, :KF])

            zfull = smallp.tile([P, KF + 1], F32, tag="zfull")
            cg_b = cg_bf.unsqueeze(1).to_broadcast((P, KF, KF))
            nc.vector.tensor_tensor(content, content, cg_b,
                                    op=mybir.AluOpType.add)
            nc.scalar.activation(content, content,
                                 mybir.ActivationFunctionType.Tanh)
            wh_b = whrep_bf.unsqueeze(1).to_broadcast((P, KF, KF))
            nc.vector.tensor_tensor(content, content, wh_b,
                                    op=mybir.AluOpType.mult)
            nc.vector.reduce_sum(zfull[:, 0:KF], content,
                                 axis=mybir.AxisListType.X)

            st.update(hid=hid, cel=cel, hT=hT, xT=xT, hpT=hpT,
                      cg=cg, zfull=zfull)
            return st

        head = produce_head(0)
        for m in range(MT):
            R0 = m * P
            hid = head["hid"]; cel = head["cel"]; hT = head["hT"]
            xT = head["xT"]; hpT = head["hpT"]; cg = head["cg"]
            zfull = head["zfull"]

            pgate = pgc.tile([P, 512], F32, tag="pgc")
            nmm = EK + HK
            i = 0
            for et in range(EK):
                nc.tensor.matmul(pgate, xT[:, et, :], WxT[:, et, :],
                                 start=(i == 0), stop=(i == nmm - 1))
                i += 1
            for kt in range(HK):
                nc.tensor.matmul(pgate, hpT[:, kt, :], WshT[:, kt, :],
                                 start=(i == 0), stop=(i == nmm - 1))
                i += 1

            sig = work.tile([P, H], F32, tag="sig")
            nc.scalar.activation(sig, pgate, mybir.ActivationFunctionType.Sigmoid)
            tnc = work.tile([P, H], F32, tag="tnc")
            nc.scalar.activation(tnc, cel, mybir.ActivationFunctionType.Tanh)
            sent = work.tile([P, H], F32, tag="sent")
            nc.vector.tensor_mul(sent, sig, tnc)

            # cs = sent@WsT + cg ; z_ext = wh . tanh(cs)
            sent_bf = work.tile([P, H], BF16, tag="sent_bf")
            nc.vector.tensor_copy(sent_bf, sent)
            sT = trp.tile([P, HK, P], BF16, tag="sT")
            ps_s = ptr.tile([P, 512], BF16, tag="ptr")
            for kt in range(HK):
                nc.tensor.transpose(
                    ps_s[:, kt * P:(kt + 1) * P],
                    sent_bf[:, kt * P:(kt + 1) * P], id_bf)
            nc.vector.tensor_copy(sT[:, :, :], ps_s)
            pcs = pgc.tile([P, 512], F32, tag="pgc")
            for kt in range(HK):
                nc.tensor.matmul(pcs[:, :KF], sT[:, kt, :], WsT[:, kt, :],
                                 start=(kt == 0), stop=(kt == HK - 1))
            cs = smallp.tile([P, KF], F32, tag="cs")
            nc.vector.tensor_add(cs, pcs[:, :KF], cg)
            tcs = smallp.tile([P, KF], F32, tag="tcs")
            nc.scalar.activation(tcs, cs, mybir.ActivationFunctionType.Tanh)
            ttmp = smallp.tile([P, KF], F32, tag="ttmp")
            nc.vector.tensor_mul(ttmp, tcs, whrep)
            nc.vector.reduce_sum(zfull[:, KF:KF + 1], ttmp,
                                 axis=mybir.AxisListType.X)

            # issue next tile's head here: PE fills the softmax/c/chat
            # serial section with next-tile transposes + cg matmuls
            if m + 1 < MT:
                head = produce_head(m + 1)

            # fixed-shift exp (shift = sum|wh|+1 bound, baked at build):
            # e49 feeds the c-matmul without waiting on the sentinel chain
            e49 = smallp.tile([P, KF], F32, tag="e49")
            s1 = smallp.tile([P, 1], F32, tag="s1")
            nc.scalar.activation(e49, zfull[:, 0:KF],
                                 mybir.ActivationFunctionType.Exp,
                                 bias=negshift, accum_out=s1)
            rs1 = smallp.tile([P, 1], F32, tag="rs1")
            nc.vector.reciprocal(rs1, s1)
            alpha_sb = smallp.tile([P, KF], F32, tag="alpha_sb")
            nc.vector.tensor_scalar_mul(alpha_sb, e49, rs1)
            nc.sync.dma_start(out=alpha_flat[R0:R0 + P, :], in_=alpha_sb)

            e_ext = smallp.tile([P, 1], F32, tag="e_ext")
            nc.scalar.activation(e_ext, zfull[:, KF:KF + 1],
                                 mybir.ActivationFunctionType.Exp,
                                 bias=negshift)
            den = smallp.tile([P, 1], F32, tag="den")
            nc.vector.tensor_add(den, s1, e_ext)
            rden = smallp.tile([P, 1], F32, tag="rden")
            nc.vector.reciprocal(rden, den)
            beta_sb = smallp.tile([P, 1], F32, tag="beta_sb")
            nc.vector.tensor_mul(beta_sb, e_ext, rden)
            nc.sync.dma_start(out=beta_flat[R0:R0 + P, :], in_=beta_sb)

            # c*s1 = e49 @ V  via block-diag pair trick
            a2 = smallp.tile([P, 2 * KF], F32, tag="a2")
            nc.vector.memset(a2, 0.0)
            nc.vector.tensor_copy(a2[0:T, 0:KF], e49[0:T, :])
            nc.vector.tensor_copy(a2[T:P, KF:2 * KF], e49[T:P, :])
            psA = ptr.tile([P, 512], F32, tag="ptr")
            nc.tensor.transpose(psA[:2 * KF, :P], a2, id_f32)
            a2T = trp.tile([2 * KF, P], BF16, tag="a2T")
            nc.vector.tensor_copy(a2T, psA[:2 * KF, :P])
            pc = pgc.tile([P, 512], F32, tag="pgc")
            nc.tensor.matmul(pc, a2T, V2[:, m, :], start=True, stop=True)

            # a = (1-beta)*(pc*rs1) + beta*sent + hid, fused:
            # rob = rs1*(1-beta); t1 = pc*rob + hid; a = sent*beta + t1
            omb = smallp.tile([P, 1], F32, tag="omb")
            nc.vector.tensor_scalar(out=omb, in0=beta_sb, scalar1=-1.0,
                                    scalar2=1.0, op0=mybir.AluOpType.mult,
                                    op1=mybir.AluOpType.add)
            rob = smallp.tile([P, 1], F32, tag="rob")
            nc.vector.tensor_mul(rob, rs1, omb)
            t1 = work.tile([P, H], F32, tag="t1")
            nc.vector.scalar_tensor_tensor(
                out=t1, in0=pc, scalar=rob, in1=hid,
                op0=mybir.AluOpType.mult, op1=mybir.AluOpType.add,
            )
            a_bf = work.tile([P, H], BF16, tag="a_bf")
            nc.vector.scalar_tensor_tensor(
                out=a_bf, in0=sent, scalar=beta_sb, in1=t1,
                op0=mybir.AluOpType.mult, op1=mybir.AluOpType.add,
            )
            aT = trp.tile([P, HK, P], BF16, tag="aT")
            ps_a = ptr.tile([P, 512], BF16, tag="ptr")
            for kt in range(HK):
                nc.tensor.transpose(
                    ps_a[:, kt * P:(kt + 1) * P],
                    a_bf[:, kt * P:(kt + 1) * P], id_bf)
            nc.vector.tensor_copy(aT[:, :, :], ps_a)

            # out = a @ WmT ; evacs alternate DVE/ACT, 7 mid-size DMAs
            for oc in range(7):
                base = oc * 1536
                width = min(1536, VOC - base)
                osb = outp.tile([P, 1536], F32, tag="osb")
                nsub = (width + 511) // 512
                for s in range(nsub):
                    nb = base + s * 512
                    w = min(512, VOC - nb)
                    po = pmain.tile([P, 512], F32, tag="pmain")
                    for kt in range(HK):
                        nc.tensor.matmul(
                            po[:, :w], aT[:, kt, :],
                            WmT[:, kt, nb:nb + w],
                            start=(kt == 0), stop=(kt == HK - 1),
                        )
                    if s % 2 == 0:
                        nc.vector.tensor_copy(osb[:, s * 512:s * 512 + w],
                                              po[:, :w])
                    else:
                        nc.scalar.copy(osb[:, s * 512:s * 512 + w], po[:, :w])
                nc.sync.dma_start(
                    out=out_flat[R0:R0 + P, base:base + width],
                    in_=osb[:, :width],
                )

    _split_multi_waits(nc)
    return nc


_GRAPH_CACHE = {}


def _get_graph(shift):
    key = round(float(shift), 3)
    if key not in _GRAPH_CACHE:
        _GRAPH_CACHE[key] = build_core_graph(key)
    return _GRAPH_CACHE[key]


def kernel(x, hiddens, cells, V, Wv, Wg, Ws, wh, Wx, Wsh, Wm, bm):
    from concourse.bass_utils import run_bass_kernel_spmd

    n_cores = 8
    B = x.shape[0]
    bc = B // n_cores

    weights = {
        "Wv": np.ascontiguousarray(Wv, np.float32),
        "Wg": np.ascontiguousarray(Wg, np.float32),
        "Ws": np.ascontiguousarray(Ws, np.float32),
        "wh": np.ascontiguousarray(wh, np.float32),
        "Wx": np.ascontiguousarray(Wx, np.float32),
        "Wsh": np.ascontiguousarray(Wsh, np.float32),
        "Wm": np.ascontiguousarray(Wm, np.float32),
    }
    in_maps = []
    for i in range(n_cores):
        sl = slice(i * bc, (i + 1) * bc)
        m = {
            "x": np.ascontiguousarray(x[sl], np.float32),
            "hiddens": np.ascontiguousarray(hiddens[sl], np.float32),
            "cells": np.ascontiguousarray(cells[sl], np.float32),
            "V": np.ascontiguousarray(V[sl], np.float32),
        }
        m.update(weights)
        in_maps.append(m)

    shift = float(np.abs(np.asarray(wh, np.float64)).sum()) + 1.0
    nc = _get_graph(shift)
    trace = bool(int(os.environ.get("KERNEL_TRACE", "0")))
    res = run_bass_kernel_spmd(nc, in_maps, core_ids=list(range(n_cores)),
                               trace=trace)
    if trace:
        kernel.last_exec_time_ns = res.exec_time_ns
        kernel.last_profile = res

    out = np.concatenate([r["out"] for r in res.results], axis=0)
    alpha = np.concatenate([r["alpha"] for r in res.results], axis=0)
    beta = np.concatenate([r["beta"] for r in res.results], axis=0)
    if np.any(bm):
        out = out + np.asarray(bm, np.float32)
    return out, alpha, beta
